# revision 16
# baseline (speedup 1.0000x reference)
"""NT-Xent / SimCLR contrastive loss on 8 Trainium2 NeuronCores.

Strategy (data-parallel over rows of the concatenated representations):
  - Host: reps = concat(z_i, z_j) -> [8192, 512], cast bf16. Core i receives
    reps rolled by -1024*i rows so its 1024 rows sit at rows 0..1023 (SPMD
    program identical on every core; positives land at col = row + 4096).
  - Device (per core), fp8 pipeline:
      phase A (per 2048-row group g, pipelined per 1024-row half h):
        load bf16 rows; 1/||row|| via fused square+rowsum (DVE) and
        exp(-0.5*ln(n2/C^2)) (ACT, one table set, = C/||row||); scale rows
        into a permuted fp8e4 staging tile (GPSIMD, strided out AP); store
        2 KiB contiguous runs to DRAM scratch; xbar DMA transpose-load
        [1024,128]u16 -> [128,1024] repsT8 half-tiles. u16 element q of a
        row packs features (2q, 2q+1) = the two DoubleRow fp8 planes.
      phase B (per group nb, m-block): sim slice via DoubleRow fp8 matmuls
        (K=512 as 2 packed 256-chunks, 2x PE rate) into [128, 2048] PSUM;
        ACT computes exp((2/C^2)*sim) with fused row-sum; DVE extracts
        self/positive diagonals with an identity mask + fused reduce.
      A(g) and B(nb=g) are interleaved so ACT/PE/DVE/GPSIMD/DMA pipeline.
      epilogue: denom = rowsum - exp(2*sim_self); partial row loss is
        ln(denom) - 2*pos; partition-sum via a ones-matmul; scalar out.
  - Host: loss = sum(core partials) / 8192.
"""

import math
import sys
import threading
from unittest import mock

sys.path.insert(0, "/opt/trn_rl_repo")

import numpy as np  # noqa: E402
import ml_dtypes  # noqa: E402

import concourse.tile as tile  # noqa: E402
from concourse import bacc, mybir  # noqa: E402
from concourse.bass_utils import run_bass_kernel_spmd  # noqa: E402
from concourse.hw_specs import get_activation_tables  # noqa: E402
from concourse.masks import make_identity  # noqa: E402
from contextlib import ExitStack  # noqa: E402

P = 128
D = 512
TWO_N = 8192
N_CORES = 8
ROWS_PER_CORE = TWO_N // N_CORES  # 1024
T_INV = 2.0  # 1 / temperature (0.5)

NB = 4  # column groups (= row groups in phase A)
CB = TWO_N // NB  # 2048 columns per group / rows per group
TPG = CB // P  # 16 [128, 512] row tiles per group
HB = 2  # halves per group (pipeline granularity)
TPH = TPG // HB  # 8 row tiles per half
HC = CB // HB  # 1024 columns per half
MB = ROWS_PER_CORE // P  # 8 m-blocks of 128 rows per core
NS = CB // 512  # 4 matmul sub-columns of 512 (one PSUM bank each)
KK = 2  # DoubleRow packed K chunks (256 features each)
LOAD_SPLIT = 8  # sub-DMAs per group load (pipeline the first tiles)

C_SCALE = 512.0  # fp8 range scale; rows stored as C*x/||x||
SIM_SCALE = T_INV / (C_SCALE * C_SCALE)  # exact power of two: 2^-17

FP32 = mybir.dt.float32
BF16 = mybir.dt.bfloat16
FP8 = mybir.dt.float8e4
U16 = mybir.dt.uint16
AF = mybir.ActivationFunctionType
ALU = mybir.AluOpType
AX = mybir.AxisListType
DR = mybir.MatmulPerfMode.DoubleRow


def _filtered_activation_tables(arch):
    """Steer every Exp/Ln/Copy activation to the one table set containing
    both Exp and Ln, so the table-load pass cannot thrash between sets."""
    tables = get_activation_tables(arch)
    target = None
    for name, funcs in tables.items():
        if AF.Exp in funcs and AF.Ln in funcs:
            target = name
            break
    if target is None:
        return tables
    steer = {AF.Exp, AF.Ln, AF.Copy, AF.Identity}
    return {
        name: (funcs if name == target else funcs - steer)
        for name, funcs in tables.items()
    }


def _build_kernel():
    nc = bacc.Bacc("TRN2", target_bir_lowering=False, debug=False,
                   num_devices=N_CORES)
    reps = nc.dram_tensor("reps", [TWO_N, D], BF16, kind="ExternalInput").ap()
    out = nc.dram_tensor("out", [1, 1], FP32, kind="ExternalOutput").ap()

    with tile.TileContext(nc) as tc, ExitStack() as ctx:
        rows_pool = ctx.enter_context(tc.tile_pool(name="rows", bufs=1))
        stag_pool = ctx.enter_context(tc.tile_pool(name="stag", bufs=2))
        sq_pool = ctx.enter_context(tc.tile_pool(name="sq", bufs=2))
        stats_pool = ctx.enter_context(tc.tile_pool(name="stats", bufs=1))
        repsT_pool = ctx.enter_context(tc.tile_pool(name="repsT", bufs=1))
        repsT0_pool = ctx.enter_context(tc.tile_pool(name="repsT0", bufs=1))
        dram_pool = ctx.enter_context(
            tc.tile_pool(name="scratch", bufs=KK * NB * HB, space="DRAM"))
        psum_pool = ctx.enter_context(
            tc.tile_pool(name="psum", bufs=2, space="PSUM"))
        exp_pool = ctx.enter_context(tc.tile_pool(name="exp", bufs=2))
        junk_pool = ctx.enter_context(tc.tile_pool(name="junk", bufs=2))
        epi_pool = ctx.enter_context(tc.tile_pool(name="epi", bufs=1))

        # --- constants -----------------------------------------------------
        ident = stats_pool.tile([P, P], FP32, tag="ident", name="ident")
        make_identity(nc, ident[:])
        ones = stats_pool.tile([P, 1], FP32, tag="ones", name="ones")
        nc.gpsimd.memset(ones[:], 1.0)

        # accumulators for the main loop
        rs_all = stats_pool.tile([P, MB * NB], FP32, tag="rs", name="rs_all")
        e_self = stats_pool.tile([P, MB], FP32, tag="eself", name="e_self")
        pos = stats_pool.tile([P, MB], FP32, tag="pos", name="pos")

        # repsT8[kk][g][h]: [128, 1024] u16 - partition p = feature pair
        # (kk*256 + 2p, kk*256 + 2p + 1) packed as 2 fp8 bytes; column q =
        # group row h*1024 + (q%8)*128 + q//8 (scratch permutation). fp8
        # bitcast views give the DoubleRow [128, 2, N] operand APs directly.
        repsT8 = [[[repsT_pool.tile([P, HC], U16, tag=f"rT{kk}_{g}_{h}",
                                    name=f"repsT8_{kk}_{g}_{h}")
                    for h in range(HB)]
                   for g in range(NB)]
                  for kk in range(KK)]

        def rhs_ap(kk, g, ns):
            v = repsT8[kk][g][ns // 2][:].bitcast(FP8).rearrange(
                "p (n two) -> p two n", two=2)
            return v[:, :, (ns % 2) * 512:(ns % 2 + 1) * 512]

        # repsT0[kk]: [128, 2*1024] fp8, plane-slab layout (plane i at cols
        # i*1024..) — LDWEIGHTS rejects the byte-interleaved stride-2 AP, so
        # deinterleave the core's own 1024 columns for the stationary side.
        repsT0 = [repsT0_pool.tile([P, 2 * ROWS_PER_CORE], FP8,
                                   tag=f"rT0_{kk}", name=f"repsT0_{kk}")
                  for kk in range(KK)]

        def lhs_ap(kk, m):
            v = repsT0[kk][:].rearrange("p (two m) -> p two m", two=2)
            return v[:, :, m * P:(m + 1) * P]

        # --- issue all row loads up front (DMA runs ahead) -----------------
        rows_g = []
        for g in range(NB):
            rg = rows_pool.tile([P, TPG * D], BF16, tag=f"rows{g}",
                                name=f"rows_{g}")
            rows_g.append(rg)
        TSUB = TPG // LOAD_SPLIT  # row tiles per sub-DMA
        for g in range(NB):
            for s in range(LOAD_SPLIT):
                r0 = g * CB + s * TSUB * P
                src = reps[r0:r0 + TSUB * P, :].rearrange(
                    "(t p) d -> p t d", p=P)
                dst = rows_g[g][:, s * TSUB * D:(s + 1) * TSUB * D].rearrange(
                    "p (t d) -> p t d", d=D)
                nc.sync.dma_start(out=dst, in_=src)

        def phase_a(g):
            # stag: permuted fp8 staging - byte addr = kk*4096 + t*256 + c
            # so each (kk, h) store is one contiguous 2 KiB run / partition.
            stag = stag_pool.tile([P, KK * TPG * 256], FP8, tag="stag",
                                  name=f"stag_{g}")
            sview = stag[:].rearrange("p (kk t c) -> p t kk c", kk=KK, c=256)
            for h in range(HB):
                n2 = stats_pool.tile([P, TPH], FP32, tag="n2", bufs=4,
                                     name=f"n2_{g}_{h}")
                for tl in range(TPH):
                    t = h * TPH + tl
                    sq = sq_pool.tile([P, D], BF16, tag="sq",
                                      name=f"sq_{g}_{t}")
                    rt = rows_g[g][:, t * D:(t + 1) * D]
                    nc.vector.scalar_tensor_tensor(
                        out=sq[:], in0=rt, scalar=1.0, in1=rt,
                        op0=ALU.mult, op1=ALU.mult,
                        accum_out=n2[:, tl:tl + 1])
                # inv = C * n2^-0.5 = exp(-0.5 * ln(n2 / C^2)); Ln+Exp share
                # one ACT table set (forced via _filtered_activation_tables).
                lnn = stats_pool.tile([P, TPH], FP32, tag="lnn", bufs=4,
                                      name=f"lnn_{g}_{h}")
                nc.scalar.activation(lnn[:], n2[:], AF.Ln,
                                     scale=1.0 / (C_SCALE * C_SCALE))
                inv = stats_pool.tile([P, TPH], FP32, tag="inv", bufs=4,
                                      name=f"inv_{g}_{h}")
                nc.scalar.activation(inv[:], lnn[:], AF.Exp, scale=-0.5)
                for tl in range(TPH):
                    t = h * TPH + tl
                    src3 = rows_g[g][:, t * D:(t + 1) * D].rearrange(
                        "p (kk c) -> p kk c", kk=KK)
                    nc.gpsimd.tensor_scalar_mul(
                        sview[:, t], src3, inv[:, tl:tl + 1])
                # store 2 KiB runs, permuted: scratch row q = p*8 + t_local
                # holds half row t_local*128 + p; transpose-load is fully
                # contiguous. repsT8 col q <-> half row (q%8)*128 + q//8.
                for kk in range(KK):
                    scr = dram_pool.tile([HC, P], U16, tag=f"scr{kk}_{g}_{h}",
                                         name=f"scr_{kk}_{g}_{h}")
                    src = stag[:, kk * TPG * 256 + h * TPH * 256:
                               kk * TPG * 256 + (h + 1) * TPH * 256]
                    nc.sync.dma_start(
                        out=scr[:].rearrange("(p t) c -> p t c", p=P),
                        in_=src.bitcast(U16).rearrange(
                            "p (t c) -> p t c", c=P))
                    nc.sync.dma_start_transpose(repsT8[kk][g][h][:], scr[:])

        # permuted column q of an et/ps tile holds group row
        # (q//1024)*1024 + (q%8)*128 + (q%1024)//8, so the columns for
        # m-block rows m*128+j (m<8 -> half 0) sit at positions 8*j + m.
        def colsel(ap_2d, m):
            return ap_2d[:, :HC].rearrange("p (j s) -> p s j", s=TPH)[:, m, :]

        def phase_b(nb):
            for m in range(MB):
                ps = psum_pool.tile([P, CB], FP32, tag="ps",
                                    name=f"ps_{nb}_{m}")
                for ns in range(NS):
                    for kk in range(KK):
                        nc.tensor.matmul(
                            ps[:, ns * 512:(ns + 1) * 512],
                            lhsT=lhs_ap(kk, m),
                            rhs=rhs_ap(kk, nb, ns),
                            start=(kk == 0), stop=(kk == KK - 1),
                            perf_mode=DR)
                et = exp_pool.tile([P, CB], BF16, tag="et",
                                   name=f"et_{nb}_{m}")
                nc.scalar.activation(
                    et[:], ps[:], AF.Exp, scale=SIM_SCALE,
                    accum_out=rs_all[:, m * NB + nb:m * NB + nb + 1])
                if nb == 0:
                    # self-similarity diagonal: own row m*128+j sits at
                    # permuted column 8*j + m of half 0.
                    junk = junk_pool.tile([P, P], FP32, tag="junk",
                                          name=f"junk_s_{m}")
                    nc.vector.scalar_tensor_tensor(
                        out=junk[:], in0=colsel(et[:], m),
                        scalar=1.0, in1=ident[:],
                        op0=ALU.mult, op1=ALU.mult,
                        accum_out=e_self[:, m:m + 1])
                if nb == 2:
                    # positive diagonal: global col 4096+row -> group 2,
                    # same permuted position 8*j + m. ps holds C^2 * sim.
                    junk = junk_pool.tile([P, P], FP32, tag="junk",
                                          name=f"junk_p_{m}")
                    nc.vector.scalar_tensor_tensor(
                        out=junk[:], in0=colsel(ps[:], m),
                        scalar=1.0, in1=ident[:],
                        op0=ALU.mult, op1=ALU.mult,
                        accum_out=pos[:, m:m + 1])

        # interleave: A(g0) B(0) | A(g1) B(1) | ... so every engine queue
        # pipelines (ACT: ln/exp(g) then 8 exps(nb); PE streams while the
        # next group loads/normalizes/transposes).
        for g in range(NB):
            phase_a(g)
            if g == 0:
                # deinterleave + unpermute the core's own 1024 columns into
                # plane-slab lhsT: fp8 index of a half-tile = 2*(8j + s) + i
                # for half row s*128 + j; own rows = half 0 of group 0.
                for kk in range(KK):
                    iv = repsT8[kk][0][0][:].bitcast(FP8).rearrange(
                        "p (j s two) -> p two s j", two=2, s=TPH)
                    ov = repsT0[kk][:].rearrange(
                        "p (two s j) -> p two s j", two=2, s=MB)
                    for i in range(2):
                        nc.vector.tensor_copy(ov[:, i], iv[:, i])
            phase_b(g)

        # --- epilogue ------------------------------------------------------
        sums = epi_pool.tile([P, MB], FP32, tag="sums", name="sums")
        nc.vector.tensor_reduce(
            sums[:], rs_all[:].rearrange("p (m b) -> p m b", b=NB),
            axis=AX.X, op=ALU.add)
        denom = epi_pool.tile([P, MB], FP32, tag="denom", name="denom")
        nc.vector.tensor_sub(denom[:], sums[:], e_self[:])
        ld = epi_pool.tile([P, MB], FP32, tag="ld", name="ld")
        nc.scalar.activation(ld[:], denom[:], AF.Ln)
        # partial = ld - (2/C^2)*pos = (pos * -SIM_SCALE) + ld
        part = epi_pool.tile([P, MB], FP32, tag="part", name="part")
        nc.vector.scalar_tensor_tensor(
            out=part[:], in0=pos[:], scalar=-SIM_SCALE, in1=ld[:],
            op0=ALU.mult, op1=ALU.add)
        rowtot = epi_pool.tile([P, 1], FP32, tag="rowtot", name="rowtot")
        nc.vector.tensor_reduce(rowtot[:], part[:], axis=AX.X, op=ALU.add)
        pfin = psum_pool.tile([P, CB], FP32, tag="ps", name="pfin")
        nc.tensor.matmul(pfin[:1, :1], lhsT=ones[:], rhs=rowtot[:])
        out_sb = epi_pool.tile([1, 1], FP32, tag="osb", name="out_sb")
        nc.vector.tensor_copy(out_sb[:], pfin[:1, :1])
        nc.sync.dma_start(out=out[:, :], in_=out_sb[:])

    with mock.patch("concourse.bacc.get_activation_tables",
                    _filtered_activation_tables):
        nc.compile()
    return nc


_CACHE_LOCK = threading.Lock()
_CACHED_NC = None


def _get_nc():
    global _CACHED_NC
    with _CACHE_LOCK:
        if _CACHED_NC is None:
            _CACHED_NC = _build_kernel()
        return _CACHED_NC


def _run(inputs, trace=False):
    z_i = np.asarray(inputs["z_i"], dtype=np.float32)
    z_j = np.asarray(inputs["z_j"], dtype=np.float32)
    reps = np.concatenate([z_i, z_j], axis=0).astype(ml_dtypes.bfloat16)
    in_maps = [
        {"reps": np.ascontiguousarray(
            np.roll(reps, -ROWS_PER_CORE * i, axis=0))}
        for i in range(N_CORES)
    ]
    nc = _get_nc()
    res = run_bass_kernel_spmd(nc, in_maps, list(range(N_CORES)), trace=trace)
    partials = [float(res.results[i]["out"][0, 0]) for i in range(N_CORES)]
    loss = np.float32(np.sum(np.asarray(partials, dtype=np.float64)) / TWO_N)
    return loss, res


def kernel(**inputs):
    loss, _ = _run(inputs, trace=False)
    return np.asarray(loss, dtype=np.float32)


# revision 17
# speedup vs baseline: 2.9444x; 2.9444x over previous
"""NT-Xent / SimCLR contrastive loss on 8 Trainium2 NeuronCores.

Strategy (data-parallel over rows of the concatenated representations):
  - Host: reps = concat(z_i, z_j) -> [8192, 512], cast bf16. Core i receives
    reps rolled by -1024*i rows so its 1024 rows sit at rows 0..1023 (SPMD
    program identical on every core; positives land at col = row + 4096).
  - Device (per core), fp8 pipeline:
      phase A (per 2048-row group g, pipelined per 1024-row half h):
        load bf16 rows; 1/||row|| via fused square+rowsum (DVE) and
        exp(-0.5*ln(n2/C^2)) (ACT, one table set, = C/||row||); scale rows
        into a permuted fp8e4 staging tile (GPSIMD, strided out AP); store
        2 KiB contiguous runs to DRAM scratch; xbar DMA transpose-load
        [1024,128]u16 -> [128,1024] repsT8 half-tiles. u16 element q of a
        row packs features (2q, 2q+1) = the two DoubleRow fp8 planes.
      phase B (per group nb, m-block): sim slice via DoubleRow fp8 matmuls
        (K=512 as 2 packed 256-chunks, 2x PE rate) into [128, 2048] PSUM;
        ACT computes exp((2/C^2)*sim) with fused row-sum; DVE extracts
        self/positive diagonals with an identity mask + fused reduce.
      A(g) and B(nb=g) are interleaved so ACT/PE/DVE/GPSIMD/DMA pipeline.
      epilogue: denom = rowsum - exp(2*sim_self); partial row loss is
        ln(denom) - 2*pos; partition-sum via a ones-matmul; scalar out.
  - Host: loss = sum(core partials) / 8192.
"""

import math
import sys
import threading
from unittest import mock

sys.path.insert(0, "/opt/trn_rl_repo")

import numpy as np  # noqa: E402
import ml_dtypes  # noqa: E402

import concourse.tile as tile  # noqa: E402
from concourse import bacc, mybir  # noqa: E402
from concourse.bass_utils import run_bass_kernel_spmd  # noqa: E402
from concourse.hw_specs import get_activation_tables  # noqa: E402
from concourse.masks import make_identity  # noqa: E402
from contextlib import ExitStack  # noqa: E402

P = 128
D = 512
TWO_N = 8192
N_CORES = 8
ROWS_PER_CORE = TWO_N // N_CORES  # 1024
T_INV = 2.0  # 1 / temperature (0.5)

NB = 4  # column groups (= row groups in phase A)
CB = TWO_N // NB  # 2048 columns per group / rows per group
TPG = CB // P  # 16 [128, 512] row tiles per group
HB = 2  # halves per group (pipeline granularity)
TPH = TPG // HB  # 8 row tiles per half
HC = CB // HB  # 1024 columns per half
MB = ROWS_PER_CORE // P  # 8 m-blocks of 128 rows per core
NS = CB // 512  # 4 matmul sub-columns of 512 (one PSUM bank each)
KK = 2  # DoubleRow packed K chunks (256 features each)
LOAD_SPLIT = 8  # sub-DMAs per group load (pipeline the first tiles)

C_SCALE = 512.0  # fp8 range scale; rows stored as C*x/||x||
SIM_SCALE = T_INV / (C_SCALE * C_SCALE)  # exact power of two: 2^-17

FP32 = mybir.dt.float32
BF16 = mybir.dt.bfloat16
FP8 = mybir.dt.float8e4
U16 = mybir.dt.uint16
AF = mybir.ActivationFunctionType
ALU = mybir.AluOpType
AX = mybir.AxisListType
DR = mybir.MatmulPerfMode.DoubleRow


def _filtered_activation_tables(arch):
    """Steer every Exp/Ln/Copy activation to the one table set containing
    both Exp and Ln, so the table-load pass cannot thrash between sets."""
    tables = get_activation_tables(arch)
    target = None
    for name, funcs in tables.items():
        if AF.Exp in funcs and AF.Ln in funcs:
            target = name
            break
    if target is None:
        return tables
    steer = {AF.Exp, AF.Ln, AF.Copy, AF.Identity}
    return {
        name: (funcs if name == target else funcs - steer)
        for name, funcs in tables.items()
    }


def _build_kernel():
    nc = bacc.Bacc("TRN2", target_bir_lowering=False, debug=False,
                   num_devices=N_CORES)
    reps = nc.dram_tensor("reps", [TWO_N, D], BF16, kind="ExternalInput").ap()
    out = nc.dram_tensor("out", [1, 1], FP32, kind="ExternalOutput").ap()

    with tile.TileContext(nc) as tc, ExitStack() as ctx:
        rows_pool = ctx.enter_context(tc.tile_pool(name="rows", bufs=1))
        stag_pool = ctx.enter_context(tc.tile_pool(name="stag", bufs=2))
        sq_pool = ctx.enter_context(tc.tile_pool(name="sq", bufs=2))
        stats_pool = ctx.enter_context(tc.tile_pool(name="stats", bufs=1))
        repsT_pool = ctx.enter_context(tc.tile_pool(name="repsT", bufs=1))
        repsT0_pool = ctx.enter_context(tc.tile_pool(name="repsT0", bufs=1))
        dram_pool = ctx.enter_context(
            tc.tile_pool(name="scratch", bufs=KK * NB * HB, space="DRAM"))
        psum_pool = ctx.enter_context(
            tc.tile_pool(name="psum", bufs=2, space="PSUM"))
        exp_pool = ctx.enter_context(tc.tile_pool(name="exp", bufs=2))
        junk_pool = ctx.enter_context(tc.tile_pool(name="junk", bufs=2))
        epi_pool = ctx.enter_context(tc.tile_pool(name="epi", bufs=1))

        # --- constants -----------------------------------------------------
        ident = stats_pool.tile([P, P], FP32, tag="ident", name="ident")
        make_identity(nc, ident[:])
        ones = stats_pool.tile([P, 1], FP32, tag="ones", name="ones")
        nc.gpsimd.memset(ones[:], 1.0)

        # accumulators for the main loop
        rs_all = stats_pool.tile([P, MB * NB], FP32, tag="rs", name="rs_all")
        e_self = stats_pool.tile([P, MB], FP32, tag="eself", name="e_self")
        pos = stats_pool.tile([P, MB], FP32, tag="pos", name="pos")

        # repsT8[kk][g][h]: [128, 1024] u16 - partition p = feature pair
        # (kk*256 + 2p, kk*256 + 2p + 1) packed as 2 fp8 bytes; column q =
        # group row h*1024 + (q%8)*128 + q//8 (scratch permutation). fp8
        # bitcast views give the DoubleRow [128, 2, N] operand APs directly.
        repsT8 = [[[repsT_pool.tile([P, HC], U16, tag=f"rT{kk}_{g}_{h}",
                                    name=f"repsT8_{kk}_{g}_{h}")
                    for h in range(HB)]
                   for g in range(NB)]
                  for kk in range(KK)]

        def rhs_ap(kk, g, ns):
            v = repsT8[kk][g][ns // 2][:].bitcast(FP8).rearrange(
                "p (n two) -> p two n", two=2)
            return v[:, :, (ns % 2) * 512:(ns % 2 + 1) * 512]

        # repsT0[kk]: [128, 2*1024] fp8, plane-slab layout (plane i at cols
        # i*1024..) — LDWEIGHTS rejects the byte-interleaved stride-2 AP, so
        # deinterleave the core's own 1024 columns for the stationary side.
        repsT0 = [repsT0_pool.tile([P, 2 * ROWS_PER_CORE], FP8,
                                   tag=f"rT0_{kk}", name=f"repsT0_{kk}")
                  for kk in range(KK)]

        def lhs_ap(kk, m):
            v = repsT0[kk][:].rearrange("p (two m) -> p two m", two=2)
            return v[:, :, m * P:(m + 1) * P]

        # --- issue all row loads up front (DMA runs ahead) -----------------
        rows_g = []
        for g in range(NB):
            rg = rows_pool.tile([P, TPG * D], BF16, tag=f"rows{g}",
                                name=f"rows_{g}")
            rows_g.append(rg)
        TSUB = TPG // LOAD_SPLIT  # row tiles per sub-DMA
        for g in range(NB):
            for s in range(LOAD_SPLIT):
                r0 = g * CB + s * TSUB * P
                src = reps[r0:r0 + TSUB * P, :].rearrange(
                    "(t p) d -> p t d", p=P)
                dst = rows_g[g][:, s * TSUB * D:(s + 1) * TSUB * D].rearrange(
                    "p (t d) -> p t d", d=D)
                nc.sync.dma_start(out=dst, in_=src)

        SQC = 4  # row tiles squared per DVE instruction

        def phase_a(g):
            normed8 = stag_pool.tile([P, TPG * D], FP8, tag="stag",
                                     name=f"normed_{g}")
            for h in range(HB):
                n2 = stats_pool.tile([P, TPH], FP32, tag="n2", bufs=4,
                                     name=f"n2_{g}_{h}")
                # big 2-port square (bf16 2x mode) + one multi-tile reduce
                # (single-src 4x mode) instead of 16 fused-accum ops.
                for c4 in range(TPH // SQC):
                    t0 = h * TPH + c4 * SQC
                    rt = rows_g[g][:, t0 * D:(t0 + SQC) * D]
                    sq = sq_pool.tile([P, SQC * D], BF16, tag="sq",
                                      name=f"sq_{g}_{t0}")
                    nc.vector.tensor_mul(sq[:], rt, rt)
                    nc.vector.tensor_reduce(
                        n2[:, c4 * SQC:(c4 + 1) * SQC],
                        sq[:].rearrange("p (t d) -> p t d", d=D),
                        axis=AX.X, op=ALU.add)
                # inv = C * n2^-0.5 = exp(-0.5 * ln(n2 / C^2)); Ln+Exp share
                # one ACT table set (forced via _filtered_activation_tables).
                lnn = stats_pool.tile([P, TPH], FP32, tag="lnn", bufs=4,
                                      name=f"lnn_{g}_{h}")
                nc.scalar.activation(lnn[:], n2[:], AF.Ln,
                                     scale=1.0 / (C_SCALE * C_SCALE))
                inv = stats_pool.tile([P, TPH], FP32, tag="inv", bufs=4,
                                      name=f"inv_{g}_{h}")
                nc.scalar.activation(inv[:], lnn[:], AF.Exp, scale=-0.5)
                for tl in range(TPH):
                    t = h * TPH + tl
                    nc.vector.tensor_scalar_mul(
                        normed8[:, t * D:(t + 1) * D],
                        rows_g[g][:, t * D:(t + 1) * D], inv[:, tl:tl + 1])
                # store, permuted: scratch row q = p*8 + t_local holds half
                # row t_local*128 + p; the transpose-load is then fully
                # contiguous. repsT8 col q <-> half row (q%8)*128 + q//8.
                nview16 = normed8[:].bitcast(U16).rearrange(
                    "p (t q) -> p t q", q=KK * P)
                for kk in range(KK):
                    scr = dram_pool.tile([HC, P], U16, tag=f"scr{kk}_{g}_{h}",
                                         name=f"scr_{kk}_{g}_{h}")
                    nc.sync.dma_start(
                        out=scr[:].rearrange("(p t) c -> p t c", p=P),
                        in_=nview16[:, h * TPH:(h + 1) * TPH,
                                    kk * P:(kk + 1) * P])
                    nc.sync.dma_start_transpose(repsT8[kk][g][h][:], scr[:])

        # permuted column q of an et/ps tile holds group row
        # (q//1024)*1024 + (q%8)*128 + (q%1024)//8, so the columns for
        # m-block rows m*128+j (m<8 -> half 0) sit at positions 8*j + m.
        def colsel(ap_2d, m):
            return ap_2d[:, :HC].rearrange("p (j s) -> p s j", s=TPH)[:, m, :]

        def phase_b(nb):
            for m in range(MB):
                ps = psum_pool.tile([P, CB], FP32, tag="ps",
                                    name=f"ps_{nb}_{m}")
                for ns in range(NS):
                    for kk in range(KK):
                        nc.tensor.matmul(
                            ps[:, ns * 512:(ns + 1) * 512],
                            lhsT=lhs_ap(kk, m),
                            rhs=rhs_ap(kk, nb, ns),
                            start=(kk == 0), stop=(kk == KK - 1),
                            perf_mode=DR)
                et = exp_pool.tile([P, CB], BF16, tag="et",
                                   name=f"et_{nb}_{m}")
                nc.scalar.activation(
                    et[:], ps[:], AF.Exp, scale=SIM_SCALE,
                    accum_out=rs_all[:, m * NB + nb:m * NB + nb + 1])
                if nb == 0:
                    # self-similarity diagonal: own row m*128+j sits at
                    # permuted column 8*j + m of half 0.
                    junk = junk_pool.tile([P, P], FP32, tag="junk",
                                          name=f"junk_s_{m}")
                    nc.vector.scalar_tensor_tensor(
                        out=junk[:], in0=colsel(et[:], m),
                        scalar=1.0, in1=ident[:],
                        op0=ALU.mult, op1=ALU.mult,
                        accum_out=e_self[:, m:m + 1])
                if nb == 2:
                    # positive diagonal: global col 4096+row -> group 2,
                    # same permuted position 8*j + m. ps holds C^2 * sim.
                    junk = junk_pool.tile([P, P], FP32, tag="junk",
                                          name=f"junk_p_{m}")
                    nc.vector.scalar_tensor_tensor(
                        out=junk[:], in0=colsel(ps[:], m),
                        scalar=1.0, in1=ident[:],
                        op0=ALU.mult, op1=ALU.mult,
                        accum_out=pos[:, m:m + 1])

        # interleave: A(g0) B(0) | A(g1) B(1) | ... so every engine queue
        # pipelines (ACT: ln/exp(g) then 8 exps(nb); PE streams while the
        # next group loads/normalizes/transposes).
        for g in range(NB):
            phase_a(g)
            if g == 0:
                # deinterleave + unpermute the core's own 1024 columns into
                # plane-slab lhsT: fp8 index of a half-tile = 2*(8j + s) + i
                # for half row s*128 + j; own rows = half 0 of group 0.
                for kk in range(KK):
                    iv = repsT8[kk][0][0][:].bitcast(FP8).rearrange(
                        "p (j s two) -> p two s j", two=2, s=TPH)
                    ov = repsT0[kk][:].rearrange(
                        "p (two s j) -> p two s j", two=2, s=MB)
                    for i in range(2):
                        nc.vector.tensor_copy(ov[:, i], iv[:, i])
            phase_b(g)

        # --- epilogue ------------------------------------------------------
        sums = epi_pool.tile([P, MB], FP32, tag="sums", name="sums")
        nc.vector.tensor_reduce(
            sums[:], rs_all[:].rearrange("p (m b) -> p m b", b=NB),
            axis=AX.X, op=ALU.add)
        denom = epi_pool.tile([P, MB], FP32, tag="denom", name="denom")
        nc.vector.tensor_sub(denom[:], sums[:], e_self[:])
        ld = epi_pool.tile([P, MB], FP32, tag="ld", name="ld")
        nc.scalar.activation(ld[:], denom[:], AF.Ln)
        # partial = ld - (2/C^2)*pos = (pos * -SIM_SCALE) + ld
        part = epi_pool.tile([P, MB], FP32, tag="part", name="part")
        nc.vector.scalar_tensor_tensor(
            out=part[:], in0=pos[:], scalar=-SIM_SCALE, in1=ld[:],
            op0=ALU.mult, op1=ALU.add)
        rowtot = epi_pool.tile([P, 1], FP32, tag="rowtot", name="rowtot")
        nc.vector.tensor_reduce(rowtot[:], part[:], axis=AX.X, op=ALU.add)
        pfin = psum_pool.tile([P, CB], FP32, tag="ps", name="pfin")
        nc.tensor.matmul(pfin[:1, :1], lhsT=ones[:], rhs=rowtot[:])
        out_sb = epi_pool.tile([1, 1], FP32, tag="osb", name="out_sb")
        nc.vector.tensor_copy(out_sb[:], pfin[:1, :1])
        nc.sync.dma_start(out=out[:, :], in_=out_sb[:])

    with mock.patch("concourse.bacc.get_activation_tables",
                    _filtered_activation_tables):
        nc.compile()
    return nc


_CACHE_LOCK = threading.Lock()
_CACHED_NC = None


def _get_nc():
    global _CACHED_NC
    with _CACHE_LOCK:
        if _CACHED_NC is None:
            _CACHED_NC = _build_kernel()
        return _CACHED_NC


def _run(inputs, trace=False):
    z_i = np.asarray(inputs["z_i"], dtype=np.float32)
    z_j = np.asarray(inputs["z_j"], dtype=np.float32)
    reps = np.concatenate([z_i, z_j], axis=0).astype(ml_dtypes.bfloat16)
    in_maps = [
        {"reps": np.ascontiguousarray(
            np.roll(reps, -ROWS_PER_CORE * i, axis=0))}
        for i in range(N_CORES)
    ]
    nc = _get_nc()
    res = run_bass_kernel_spmd(nc, in_maps, list(range(N_CORES)), trace=trace)
    partials = [float(res.results[i]["out"][0, 0]) for i in range(N_CORES)]
    loss = np.float32(np.sum(np.asarray(partials, dtype=np.float64)) / TWO_N)
    return loss, res


def kernel(**inputs):
    loss, _ = _run(inputs, trace=False)
    return np.asarray(loss, dtype=np.float32)


# revision 19
# speedup vs baseline: 3.7822x; 1.2845x over previous
"""NT-Xent / SimCLR contrastive loss on 8 Trainium2 NeuronCores.

Strategy (data-parallel over rows of the concatenated representations):
  - Host: reps = concat(z_i, z_j) -> [8192, 512], cast bf16. Core i receives
    reps rolled by -1024*i rows so its 1024 rows sit at rows 0..1023 (SPMD
    program identical on every core; positives land at col = row + 4096).
  - Device (per core), fp8 pipeline:
      phase A (per 2048-row group g, pipelined per 1024-row half h):
        load bf16 rows; 1/||row|| via fused square+rowsum (DVE) and
        exp(-0.5*ln(n2/C^2)) (ACT, one table set, = C/||row||); scale rows
        into a permuted fp8e4 staging tile (GPSIMD, strided out AP); store
        2 KiB contiguous runs to DRAM scratch; xbar DMA transpose-load
        [1024,128]u16 -> [128,1024] repsT8 half-tiles. u16 element q of a
        row packs features (2q, 2q+1) = the two DoubleRow fp8 planes.
      phase B (per group nb, m-block): sim slice via DoubleRow fp8 matmuls
        (K=512 as 2 packed 256-chunks, 2x PE rate) into [128, 2048] PSUM;
        ACT computes exp((2/C^2)*sim) with fused row-sum; DVE extracts
        self/positive diagonals with an identity mask + fused reduce.
      A(g) and B(nb=g) are interleaved so ACT/PE/DVE/GPSIMD/DMA pipeline.
      epilogue: denom = rowsum - exp(2*sim_self); partial row loss is
        ln(denom) - 2*pos; partition-sum via a ones-matmul; scalar out.
  - Host: loss = sum(core partials) / 8192.
"""

import math
import sys
import threading
from unittest import mock

sys.path.insert(0, "/opt/trn_rl_repo")

import numpy as np  # noqa: E402
import ml_dtypes  # noqa: E402

import concourse.tile as tile  # noqa: E402
from concourse import bacc, mybir  # noqa: E402
from concourse.bass_utils import run_bass_kernel_spmd  # noqa: E402
from concourse.hw_specs import get_activation_tables  # noqa: E402
from concourse.masks import make_identity  # noqa: E402
from contextlib import ExitStack  # noqa: E402

P = 128
D = 512
TWO_N = 8192
N_CORES = 8
ROWS_PER_CORE = TWO_N // N_CORES  # 1024
T_INV = 2.0  # 1 / temperature (0.5)

NB = 4  # column groups (= row groups in phase A)
CB = TWO_N // NB  # 2048 columns per group / rows per group
TPG = CB // P  # 16 [128, 512] row tiles per group
HB = 2  # halves per group (pipeline granularity)
TPH = TPG // HB  # 8 row tiles per half
HC = CB // HB  # 1024 columns per half
MB = ROWS_PER_CORE // P  # 8 m-blocks of 128 rows per core
NS = CB // 512  # 4 matmul sub-columns of 512 (one PSUM bank each)
KK = 2  # DoubleRow packed K chunks (256 features each)
LOAD_SPLIT = 8  # sub-DMAs per group load (pipeline the first tiles)

C_SCALE = 512.0  # fp8 range scale; rows stored as C*x/||x||
SIM_SCALE = T_INV / (C_SCALE * C_SCALE)  # exact power of two: 2^-17

FP32 = mybir.dt.float32
BF16 = mybir.dt.bfloat16
FP8 = mybir.dt.float8e4
U16 = mybir.dt.uint16
AF = mybir.ActivationFunctionType
ALU = mybir.AluOpType
AX = mybir.AxisListType
DR = mybir.MatmulPerfMode.DoubleRow


def _filtered_activation_tables(arch):
    """Steer every Exp/Ln/Copy activation to the one table set containing
    both Exp and Ln, so the table-load pass cannot thrash between sets."""
    tables = get_activation_tables(arch)
    target = None
    for name, funcs in tables.items():
        if AF.Exp in funcs and AF.Ln in funcs:
            target = name
            break
    if target is None:
        return tables
    steer = {AF.Exp, AF.Ln, AF.Copy, AF.Identity}
    return {
        name: (funcs if name == target else funcs - steer)
        for name, funcs in tables.items()
    }


def _build_kernel():
    nc = bacc.Bacc("TRN2", target_bir_lowering=False, debug=False,
                   num_devices=N_CORES)
    reps = nc.dram_tensor("reps", [TWO_N, D], BF16, kind="ExternalInput").ap()
    out = nc.dram_tensor("out", [1, 1], FP32, kind="ExternalOutput").ap()

    with tile.TileContext(nc) as tc, ExitStack() as ctx:
        rows_pool = ctx.enter_context(tc.tile_pool(name="rows", bufs=1))
        stag_pool = ctx.enter_context(tc.tile_pool(name="stag", bufs=2))
        sq_pool = ctx.enter_context(tc.tile_pool(name="sq", bufs=2))
        stats_pool = ctx.enter_context(tc.tile_pool(name="stats", bufs=1))
        repsT_pool = ctx.enter_context(tc.tile_pool(name="repsT", bufs=1))
        repsT0_pool = ctx.enter_context(tc.tile_pool(name="repsT0", bufs=1))
        dram_pool = ctx.enter_context(
            tc.tile_pool(name="scratch", bufs=KK * NB * HB, space="DRAM"))
        psum_pool = ctx.enter_context(
            tc.tile_pool(name="psum", bufs=2, space="PSUM"))
        exp_pool = ctx.enter_context(tc.tile_pool(name="exp", bufs=2))
        junk_pool = ctx.enter_context(tc.tile_pool(name="junk", bufs=2))
        epi_pool = ctx.enter_context(tc.tile_pool(name="epi", bufs=1))

        # --- constants -----------------------------------------------------
        ident = stats_pool.tile([P, P], FP32, tag="ident", name="ident")
        make_identity(nc, ident[:])
        ones = stats_pool.tile([P, 1], FP32, tag="ones", name="ones")
        nc.gpsimd.memset(ones[:], 1.0)

        # accumulators for the main loop
        rs_all = stats_pool.tile([P, MB * NB], FP32, tag="rs", name="rs_all")
        e_self = stats_pool.tile([P, MB], FP32, tag="eself", name="e_self")
        pos = stats_pool.tile([P, MB], FP32, tag="pos", name="pos")

        # repsT8[kk][g][h]: [128, 1024] u16 - partition p = feature pair
        # (kk*256 + 2p, kk*256 + 2p + 1) packed as 2 fp8 bytes; column q =
        # group row h*1024 + (q%8)*128 + q//8 (scratch permutation). fp8
        # bitcast views give the DoubleRow [128, 2, N] operand APs directly.
        repsT8 = [[[repsT_pool.tile([P, HC], U16, tag=f"rT{kk}_{g}_{h}",
                                    name=f"repsT8_{kk}_{g}_{h}")
                    for h in range(HB)]
                   for g in range(NB)]
                  for kk in range(KK)]

        def rhs_ap(kk, g, ns):
            v = repsT8[kk][g][ns // 2][:].bitcast(FP8).rearrange(
                "p (n two) -> p two n", two=2)
            return v[:, :, (ns % 2) * 512:(ns % 2 + 1) * 512]

        # repsT0[kk]: [128, 2*1024] fp8, plane-slab layout (plane i at cols
        # i*1024..) — LDWEIGHTS rejects the byte-interleaved stride-2 AP, so
        # deinterleave the core's own 1024 columns for the stationary side.
        repsT0 = [repsT0_pool.tile([P, 2 * ROWS_PER_CORE], FP8,
                                   tag=f"rT0_{kk}", name=f"repsT0_{kk}")
                  for kk in range(KK)]

        def lhs_ap(kk, m):
            v = repsT0[kk][:].rearrange("p (two m) -> p two m", two=2)
            return v[:, :, m * P:(m + 1) * P]

        # --- issue all row loads up front (DMA runs ahead) -----------------
        rows_g = []
        for g in range(NB):
            rg = rows_pool.tile([P, TPG * D], BF16, tag=f"rows{g}",
                                name=f"rows_{g}")
            rows_g.append(rg)
        TSUB = TPG // LOAD_SPLIT  # row tiles per sub-DMA
        for g in range(NB):
            for s in range(LOAD_SPLIT):
                r0 = g * CB + s * TSUB * P
                src = reps[r0:r0 + TSUB * P, :].rearrange(
                    "(t p) d -> p t d", p=P)
                dst = rows_g[g][:, s * TSUB * D:(s + 1) * TSUB * D].rearrange(
                    "p (t d) -> p t d", d=D)
                nc.sync.dma_start(out=dst, in_=src)

        def phase_a(g):
            # stag: permuted fp8 staging - byte addr = kk*4096 + t*256 + c -
            # so each (kk, h) store is one contiguous 2 KiB run / partition
            # (128 descriptors instead of 1024). DVE writes it with a
            # strided out AP (DVE runs 1x regardless; strides are free).
            stag = stag_pool.tile([P, KK * TPG * 256], FP8, tag="stag",
                                  name=f"stag_{g}")
            sview = stag[:].rearrange("p (kk t c) -> p t kk c", kk=KK, c=256)
            for h in range(HB):
                n2 = stats_pool.tile([P, TPH], FP32, tag="n2", bufs=4,
                                     name=f"n2_{g}_{h}")
                for tl in range(TPH):
                    t = h * TPH + tl
                    sq = sq_pool.tile([P, D], BF16, tag="sq",
                                      name=f"sq_{g}_{t}")
                    rt = rows_g[g][:, t * D:(t + 1) * D]
                    nc.vector.scalar_tensor_tensor(
                        out=sq[:], in0=rt, scalar=1.0, in1=rt,
                        op0=ALU.mult, op1=ALU.mult,
                        accum_out=n2[:, tl:tl + 1])
                # inv = C * n2^-0.5 = exp(-0.5 * ln(n2 / C^2)); Ln+Exp share
                # one ACT table set (forced via _filtered_activation_tables).
                lnn = stats_pool.tile([P, TPH], FP32, tag="lnn", bufs=4,
                                      name=f"lnn_{g}_{h}")
                nc.scalar.activation(lnn[:], n2[:], AF.Ln,
                                     scale=1.0 / (C_SCALE * C_SCALE))
                inv = stats_pool.tile([P, TPH], FP32, tag="inv", bufs=4,
                                      name=f"inv_{g}_{h}")
                nc.scalar.activation(inv[:], lnn[:], AF.Exp, scale=-0.5)
                for tl in range(TPH):
                    t = h * TPH + tl
                    src3 = rows_g[g][:, t * D:(t + 1) * D].rearrange(
                        "p (kk c) -> p kk c", kk=KK)
                    nc.vector.tensor_scalar_mul(
                        sview[:, t], src3, inv[:, tl:tl + 1])
                # store 2 KiB runs, permuted: scratch row q = p*8 + t_local
                # holds half row t_local*128 + p; the transpose-load is then
                # fully contiguous. repsT8 col q <-> half row (q%8)*128+q//8.
                for kk in range(KK):
                    scr = dram_pool.tile([HC, P], U16, tag=f"scr{kk}_{g}_{h}",
                                         name=f"scr_{kk}_{g}_{h}")
                    src = stag[:, kk * TPG * 256 + h * TPH * 256:
                               kk * TPG * 256 + (h + 1) * TPH * 256]
                    nc.sync.dma_start(
                        out=scr[:].rearrange("(p t) c -> p t c", p=P),
                        in_=src.bitcast(U16).rearrange(
                            "p (t c) -> p t c", c=P))
                    nc.sync.dma_start_transpose(repsT8[kk][g][h][:], scr[:])

        # permuted column q of an et/ps tile holds group row
        # (q//1024)*1024 + (q%8)*128 + (q%1024)//8, so the columns for
        # m-block rows m*128+j (m<8 -> half 0) sit at positions 8*j + m.
        def colsel(ap_2d, m):
            return ap_2d[:, :HC].rearrange("p (j s) -> p s j", s=TPH)[:, m, :]

        def phase_b(nb):
            for m in range(MB):
                ps = psum_pool.tile([P, CB], FP32, tag="ps",
                                    name=f"ps_{nb}_{m}")
                for ns in range(NS):
                    for kk in range(KK):
                        nc.tensor.matmul(
                            ps[:, ns * 512:(ns + 1) * 512],
                            lhsT=lhs_ap(kk, m),
                            rhs=rhs_ap(kk, nb, ns),
                            start=(kk == 0), stop=(kk == KK - 1),
                            perf_mode=DR)
                et = exp_pool.tile([P, CB], BF16, tag="et",
                                   name=f"et_{nb}_{m}")
                nc.scalar.activation(
                    et[:], ps[:], AF.Exp, scale=SIM_SCALE,
                    accum_out=rs_all[:, m * NB + nb:m * NB + nb + 1])
                if nb == 0:
                    # self-similarity diagonal: own row m*128+j sits at
                    # permuted column 8*j + m of half 0.
                    junk = junk_pool.tile([P, P], FP32, tag="junk",
                                          name=f"junk_s_{m}")
                    nc.vector.scalar_tensor_tensor(
                        out=junk[:], in0=colsel(et[:], m),
                        scalar=1.0, in1=ident[:],
                        op0=ALU.mult, op1=ALU.mult,
                        accum_out=e_self[:, m:m + 1])
                if nb == 2:
                    # positive diagonal: global col 4096+row -> group 2,
                    # same permuted position 8*j + m. ps holds C^2 * sim.
                    junk = junk_pool.tile([P, P], FP32, tag="junk",
                                          name=f"junk_p_{m}")
                    nc.vector.scalar_tensor_tensor(
                        out=junk[:], in0=colsel(ps[:], m),
                        scalar=1.0, in1=ident[:],
                        op0=ALU.mult, op1=ALU.mult,
                        accum_out=pos[:, m:m + 1])

        # interleave: A(g0) B(0) | A(g1) B(1) | ... so every engine queue
        # pipelines (ACT: ln/exp(g) then 8 exps(nb); PE streams while the
        # next group loads/normalizes/transposes).
        for g in range(NB):
            phase_a(g)
            if g == 0:
                # deinterleave + unpermute the core's own 1024 columns into
                # plane-slab lhsT: fp8 index of a half-tile = 2*(8j + s) + i
                # for half row s*128 + j; own rows = half 0 of group 0.
                # ACT does these strided copies: it is idle in the prefix
                # and Copy lives in every table set (no table thrash).
                for kk in range(KK):
                    iv = repsT8[kk][0][0][:].bitcast(FP8).rearrange(
                        "p (j s two) -> p two s j", two=2, s=TPH)
                    ov = repsT0[kk][:].rearrange(
                        "p (two s j) -> p two s j", two=2, s=MB)
                    for i in range(2):
                        nc.scalar.activation(ov[:, i], iv[:, i], AF.Copy)
            phase_b(g)

        # --- epilogue ------------------------------------------------------
        sums = epi_pool.tile([P, MB], FP32, tag="sums", name="sums")
        nc.vector.tensor_reduce(
            sums[:], rs_all[:].rearrange("p (m b) -> p m b", b=NB),
            axis=AX.X, op=ALU.add)
        denom = epi_pool.tile([P, MB], FP32, tag="denom", name="denom")
        nc.vector.tensor_sub(denom[:], sums[:], e_self[:])
        ld = epi_pool.tile([P, MB], FP32, tag="ld", name="ld")
        nc.scalar.activation(ld[:], denom[:], AF.Ln)
        # partial = ld - (2/C^2)*pos = (pos * -SIM_SCALE) + ld
        part = epi_pool.tile([P, MB], FP32, tag="part", name="part")
        nc.vector.scalar_tensor_tensor(
            out=part[:], in0=pos[:], scalar=-SIM_SCALE, in1=ld[:],
            op0=ALU.mult, op1=ALU.add)
        rowtot = epi_pool.tile([P, 1], FP32, tag="rowtot", name="rowtot")
        nc.vector.tensor_reduce(rowtot[:], part[:], axis=AX.X, op=ALU.add)
        pfin = psum_pool.tile([P, CB], FP32, tag="ps", name="pfin")
        nc.tensor.matmul(pfin[:1, :1], lhsT=ones[:], rhs=rowtot[:])
        out_sb = epi_pool.tile([1, 1], FP32, tag="osb", name="out_sb")
        nc.vector.tensor_copy(out_sb[:], pfin[:1, :1])
        nc.sync.dma_start(out=out[:, :], in_=out_sb[:])

    with mock.patch("concourse.bacc.get_activation_tables",
                    _filtered_activation_tables):
        nc.compile()
    return nc


_CACHE_LOCK = threading.Lock()
_CACHED_NC = None


def _get_nc():
    global _CACHED_NC
    with _CACHE_LOCK:
        if _CACHED_NC is None:
            _CACHED_NC = _build_kernel()
        return _CACHED_NC


def _run(inputs, trace=False):
    z_i = np.asarray(inputs["z_i"], dtype=np.float32)
    z_j = np.asarray(inputs["z_j"], dtype=np.float32)
    reps = np.concatenate([z_i, z_j], axis=0).astype(ml_dtypes.bfloat16)
    in_maps = [
        {"reps": np.ascontiguousarray(
            np.roll(reps, -ROWS_PER_CORE * i, axis=0))}
        for i in range(N_CORES)
    ]
    nc = _get_nc()
    res = run_bass_kernel_spmd(nc, in_maps, list(range(N_CORES)), trace=trace)
    partials = [float(res.results[i]["out"][0, 0]) for i in range(N_CORES)]
    loss = np.float32(np.sum(np.asarray(partials, dtype=np.float64)) / TWO_N)
    return loss, res


def kernel(**inputs):
    loss, _ = _run(inputs, trace=False)
    return np.asarray(loss, dtype=np.float32)


# revision 20
# speedup vs baseline: 3.8609x; 1.0208x over previous
"""NT-Xent / SimCLR contrastive loss on 8 Trainium2 NeuronCores.

Strategy (data-parallel over rows of the concatenated representations):
  - Host: reps = concat(z_i, z_j) -> [8192, 512], cast bf16. Core i receives
    reps rolled by -1024*i rows so its 1024 rows sit at rows 0..1023 (SPMD
    program identical on every core; positives land at col = row + 4096).
  - Device (per core), fp8 pipeline:
      phase A (per 2048-row group g, pipelined per 1024-row half h):
        load bf16 rows; 1/||row|| via fused square+rowsum (DVE) and
        exp(-0.5*ln(n2/C^2)) (ACT, one table set, = C/||row||); scale rows
        into a permuted fp8e4 staging tile (GPSIMD, strided out AP); store
        2 KiB contiguous runs to DRAM scratch; xbar DMA transpose-load
        [1024,128]u16 -> [128,1024] repsT8 half-tiles. u16 element q of a
        row packs features (2q, 2q+1) = the two DoubleRow fp8 planes.
      phase B (per group nb, m-block): sim slice via DoubleRow fp8 matmuls
        (K=512 as 2 packed 256-chunks, 2x PE rate) into [128, 2048] PSUM;
        ACT computes exp((2/C^2)*sim) with fused row-sum; DVE extracts
        self/positive diagonals with an identity mask + fused reduce.
      A(g) and B(nb=g) are interleaved so ACT/PE/DVE/GPSIMD/DMA pipeline.
      epilogue: denom = rowsum - exp(2*sim_self); partial row loss is
        ln(denom) - 2*pos; partition-sum via a ones-matmul; scalar out.
  - Host: loss = sum(core partials) / 8192.
"""

import math
import sys
import threading
from unittest import mock

sys.path.insert(0, "/opt/trn_rl_repo")

import numpy as np  # noqa: E402
import ml_dtypes  # noqa: E402

import concourse.tile as tile  # noqa: E402
from concourse import bacc, mybir  # noqa: E402
from concourse.bass_utils import run_bass_kernel_spmd  # noqa: E402
from concourse.hw_specs import get_activation_tables  # noqa: E402
from concourse.masks import make_identity  # noqa: E402
from contextlib import ExitStack  # noqa: E402

P = 128
D = 512
TWO_N = 8192
N_CORES = 8
ROWS_PER_CORE = TWO_N // N_CORES  # 1024
T_INV = 2.0  # 1 / temperature (0.5)

NB = 4  # column groups (= row groups in phase A)
CB = TWO_N // NB  # 2048 columns per group / rows per group
TPG = CB // P  # 16 [128, 512] row tiles per group
HB = 2  # halves per group (pipeline granularity)
TPH = TPG // HB  # 8 row tiles per half
HC = CB // HB  # 1024 columns per half
MB = ROWS_PER_CORE // P  # 8 m-blocks of 128 rows per core
NS = CB // 512  # 4 matmul sub-columns of 512 (one PSUM bank each)
KK = 2  # DoubleRow packed K chunks (256 features each)
LOAD_SPLIT = 8  # sub-DMAs per group load (pipeline the first tiles)

C_SCALE = 512.0  # fp8 range scale; rows stored as C*x/||x||
SIM_SCALE = T_INV / (C_SCALE * C_SCALE)  # exact power of two: 2^-17

FP32 = mybir.dt.float32
BF16 = mybir.dt.bfloat16
FP8 = mybir.dt.float8e4
U16 = mybir.dt.uint16
AF = mybir.ActivationFunctionType
ALU = mybir.AluOpType
AX = mybir.AxisListType
DR = mybir.MatmulPerfMode.DoubleRow


def _filtered_activation_tables(arch):
    """Steer every Exp/Ln/Copy activation to the one table set containing
    both Exp and Ln, so the table-load pass cannot thrash between sets."""
    tables = get_activation_tables(arch)
    target = None
    for name, funcs in tables.items():
        if AF.Exp in funcs and AF.Ln in funcs:
            target = name
            break
    if target is None:
        return tables
    steer = {AF.Exp, AF.Ln, AF.Copy, AF.Identity}
    return {
        name: (funcs if name == target else funcs - steer)
        for name, funcs in tables.items()
    }


def _build_kernel():
    nc = bacc.Bacc("TRN2", target_bir_lowering=False, debug=False,
                   num_devices=N_CORES)
    reps = nc.dram_tensor("reps", [TWO_N, D], BF16, kind="ExternalInput").ap()
    out = nc.dram_tensor("out", [1, 1], FP32, kind="ExternalOutput").ap()

    with tile.TileContext(nc) as tc, ExitStack() as ctx:
        rows_pool = ctx.enter_context(tc.tile_pool(name="rows", bufs=1))
        stag_pool = ctx.enter_context(tc.tile_pool(name="stag", bufs=2))
        sq_pool = ctx.enter_context(tc.tile_pool(name="sq", bufs=2))
        stats_pool = ctx.enter_context(tc.tile_pool(name="stats", bufs=1))
        repsT_pool = ctx.enter_context(tc.tile_pool(name="repsT", bufs=1))
        repsT0_pool = ctx.enter_context(tc.tile_pool(name="repsT0", bufs=1))
        dram_pool = ctx.enter_context(
            tc.tile_pool(name="scratch", bufs=KK * NB * HB, space="DRAM"))
        psum_pool = ctx.enter_context(
            tc.tile_pool(name="psum", bufs=2, space="PSUM"))
        exp_pool = ctx.enter_context(tc.tile_pool(name="exp", bufs=3))
        junk_pool = ctx.enter_context(tc.tile_pool(name="junk", bufs=2))
        epi_pool = ctx.enter_context(tc.tile_pool(name="epi", bufs=1))

        # --- constants -----------------------------------------------------
        ident = stats_pool.tile([P, P], FP32, tag="ident", name="ident")
        make_identity(nc, ident[:])
        ones = stats_pool.tile([P, 1], FP32, tag="ones", name="ones")
        nc.gpsimd.memset(ones[:], 1.0)

        # accumulators for the main loop
        rs_all = stats_pool.tile([P, MB * NB], FP32, tag="rs", name="rs_all")
        e_self = stats_pool.tile([P, MB], FP32, tag="eself", name="e_self")
        pos = stats_pool.tile([P, MB], FP32, tag="pos", name="pos")

        # repsT8[kk][g][h]: [128, 1024] u16 - partition p = feature pair
        # (kk*256 + 2p, kk*256 + 2p + 1) packed as 2 fp8 bytes; column q =
        # group row h*1024 + (q%8)*128 + q//8 (scratch permutation). fp8
        # bitcast views give the DoubleRow [128, 2, N] operand APs directly.
        repsT8 = [[[repsT_pool.tile([P, HC], U16, tag=f"rT{kk}_{g}_{h}",
                                    name=f"repsT8_{kk}_{g}_{h}")
                    for h in range(HB)]
                   for g in range(NB)]
                  for kk in range(KK)]

        def rhs_ap(kk, g, ns):
            v = repsT8[kk][g][ns // 2][:].bitcast(FP8).rearrange(
                "p (n two) -> p two n", two=2)
            return v[:, :, (ns % 2) * 512:(ns % 2 + 1) * 512]

        # repsT0[kk]: [128, 2*1024] fp8, plane-slab layout (plane i at cols
        # i*1024..) — LDWEIGHTS rejects the byte-interleaved stride-2 AP, so
        # deinterleave the core's own 1024 columns for the stationary side.
        repsT0 = [repsT0_pool.tile([P, 2 * ROWS_PER_CORE], FP8,
                                   tag=f"rT0_{kk}", name=f"repsT0_{kk}")
                  for kk in range(KK)]

        def lhs_ap(kk, m):
            v = repsT0[kk][:].rearrange("p (two m) -> p two m", two=2)
            return v[:, :, m * P:(m + 1) * P]

        # --- issue all row loads up front (DMA runs ahead) -----------------
        # one tile per (g, h) so consumers wait per-half, not per-group
        # (tile-granular dependency tracking).
        rows_gh = [[rows_pool.tile([P, TPH * D], BF16, tag=f"rows{g}_{h}",
                                   name=f"rows_{g}_{h}")
                    for h in range(HB)]
                   for g in range(NB)]
        SSUB = 2  # sub-DMAs per half-load
        TSUB = TPH // SSUB
        for g in range(NB):
            for h in range(HB):
                for s in range(SSUB):
                    r0 = g * CB + h * HC + s * TSUB * P
                    src = reps[r0:r0 + TSUB * P, :].rearrange(
                        "(t p) d -> p t d", p=P)
                    dst = rows_gh[g][h][
                        :, s * TSUB * D:(s + 1) * TSUB * D].rearrange(
                        "p (t d) -> p t d", d=D)
                    nc.sync.dma_start(out=dst, in_=src)

        def phase_a(g):
            for h in range(HB):
                rows_h = rows_gh[g][h]
                n2 = stats_pool.tile([P, TPH], FP32, tag="n2", bufs=4,
                                     name=f"n2_{g}_{h}")
                for tl in range(TPH):
                    sq = sq_pool.tile([P, D], BF16, tag="sq",
                                      name=f"sq_{g}_{h}_{tl}")
                    rt = rows_h[:, tl * D:(tl + 1) * D]
                    nc.vector.scalar_tensor_tensor(
                        out=sq[:], in0=rt, scalar=1.0, in1=rt,
                        op0=ALU.mult, op1=ALU.mult,
                        accum_out=n2[:, tl:tl + 1])
                # inv = C * n2^-0.5 = exp(-0.5 * ln(n2 / C^2)); Ln+Exp share
                # one ACT table set (forced via _filtered_activation_tables).
                lnn = stats_pool.tile([P, TPH], FP32, tag="lnn", bufs=4,
                                      name=f"lnn_{g}_{h}")
                nc.scalar.activation(lnn[:], n2[:], AF.Ln,
                                     scale=1.0 / (C_SCALE * C_SCALE))
                inv = stats_pool.tile([P, TPH], FP32, tag="inv", bufs=4,
                                      name=f"inv_{g}_{h}")
                nc.scalar.activation(inv[:], lnn[:], AF.Exp, scale=-0.5)
                # stag: permuted fp8 staging (one tile per half) - byte addr
                # = kk*2048 + t*256 + c - so each kk store is one contiguous
                # 2 KiB run per partition (128 descriptors, not 1024). DVE
                # writes it with a strided out AP (DVE is 1x regardless).
                stag = stag_pool.tile([P, KK * TPH * 256], FP8, tag="stag",
                                      bufs=3, name=f"stag_{g}_{h}")
                sview = stag[:].rearrange("p (kk t c) -> p t kk c",
                                          kk=KK, c=256)
                for tl in range(TPH):
                    src3 = rows_h[:, tl * D:(tl + 1) * D].rearrange(
                        "p (kk c) -> p kk c", kk=KK)
                    nc.vector.tensor_scalar_mul(
                        sview[:, tl], src3, inv[:, tl:tl + 1])
                # store 2 KiB runs, permuted: scratch row q = p*8 + t_local
                # holds half row t_local*128 + p; the transpose-load is then
                # fully contiguous. repsT8 col q <-> half row (q%8)*128+q//8.
                for kk in range(KK):
                    scr = dram_pool.tile([HC, P], U16, tag=f"scr{kk}_{g}_{h}",
                                         name=f"scr_{kk}_{g}_{h}")
                    src = stag[:, kk * TPH * 256:(kk + 1) * TPH * 256]
                    nc.sync.dma_start(
                        out=scr[:].rearrange("(p t) c -> p t c", p=P),
                        in_=src.bitcast(U16).rearrange(
                            "p (t c) -> p t c", c=P))
                    nc.sync.dma_start_transpose(repsT8[kk][g][h][:], scr[:])

        # permuted column q of an et/ps tile holds group row
        # (q//1024)*1024 + (q%8)*128 + (q%1024)//8, so the columns for
        # m-block rows m*128+j (m<8 -> half 0) sit at positions 8*j + m.
        def colsel(ap_2d, m):
            return ap_2d[:, :HC].rearrange("p (j s) -> p s j", s=TPH)[:, m, :]

        def phase_b(nb):
            for m in range(MB):
                ps = psum_pool.tile([P, CB], FP32, tag="ps",
                                    name=f"ps_{nb}_{m}")
                for ns in range(NS):
                    for kk in range(KK):
                        nc.tensor.matmul(
                            ps[:, ns * 512:(ns + 1) * 512],
                            lhsT=lhs_ap(kk, m),
                            rhs=rhs_ap(kk, nb, ns),
                            start=(kk == 0), stop=(kk == KK - 1),
                            perf_mode=DR)
                et = exp_pool.tile([P, CB], BF16, tag="et",
                                   name=f"et_{nb}_{m}")
                nc.scalar.activation(
                    et[:], ps[:], AF.Exp, scale=SIM_SCALE,
                    accum_out=rs_all[:, m * NB + nb:m * NB + nb + 1])
                if nb == 0:
                    # self-similarity diagonal: own row m*128+j sits at
                    # permuted column 8*j + m of half 0.
                    junk = junk_pool.tile([P, P], FP32, tag="junk",
                                          name=f"junk_s_{m}")
                    nc.vector.scalar_tensor_tensor(
                        out=junk[:], in0=colsel(et[:], m),
                        scalar=1.0, in1=ident[:],
                        op0=ALU.mult, op1=ALU.mult,
                        accum_out=e_self[:, m:m + 1])
                if nb == 2:
                    # positive diagonal: global col 4096+row -> group 2,
                    # same permuted position 8*j + m. ps holds C^2 * sim.
                    junk = junk_pool.tile([P, P], FP32, tag="junk",
                                          name=f"junk_p_{m}")
                    nc.vector.scalar_tensor_tensor(
                        out=junk[:], in0=colsel(ps[:], m),
                        scalar=1.0, in1=ident[:],
                        op0=ALU.mult, op1=ALU.mult,
                        accum_out=pos[:, m:m + 1])

        # interleave: A(g0) B(0) | A(g1) B(1) | ... so every engine queue
        # pipelines (ACT: ln/exp(g) then 8 exps(nb); PE streams while the
        # next group loads/normalizes/transposes).
        for g in range(NB):
            phase_a(g)
            if g > 0:
                phase_b(g - 1)
            if g == 0:
                # deinterleave + unpermute the core's own 1024 columns into
                # plane-slab lhsT: fp8 index of a half-tile = 2*(8j + s) + i
                # for half row s*128 + j; own rows = half 0 of group 0.
                # ACT does these strided copies: it is idle in the prefix
                # and Copy lives in every table set (no table thrash).
                for kk in range(KK):
                    iv = repsT8[kk][0][0][:].bitcast(FP8).rearrange(
                        "p (j s two) -> p two s j", two=2, s=TPH)
                    ov = repsT0[kk][:].rearrange(
                        "p (two s j) -> p two s j", two=2, s=MB)
                    for i in range(2):
                        nc.scalar.activation(ov[:, i], iv[:, i], AF.Copy)
                phase_b(0)
        phase_b(NB - 1)

        # --- epilogue ------------------------------------------------------
        sums = epi_pool.tile([P, MB], FP32, tag="sums", name="sums")
        nc.vector.tensor_reduce(
            sums[:], rs_all[:].rearrange("p (m b) -> p m b", b=NB),
            axis=AX.X, op=ALU.add)
        denom = epi_pool.tile([P, MB], FP32, tag="denom", name="denom")
        nc.vector.tensor_sub(denom[:], sums[:], e_self[:])
        ld = epi_pool.tile([P, MB], FP32, tag="ld", name="ld")
        nc.scalar.activation(ld[:], denom[:], AF.Ln)
        # partial = ld - (2/C^2)*pos = (pos * -SIM_SCALE) + ld
        part = epi_pool.tile([P, MB], FP32, tag="part", name="part")
        nc.vector.scalar_tensor_tensor(
            out=part[:], in0=pos[:], scalar=-SIM_SCALE, in1=ld[:],
            op0=ALU.mult, op1=ALU.add)
        rowtot = epi_pool.tile([P, 1], FP32, tag="rowtot", name="rowtot")
        nc.vector.tensor_reduce(rowtot[:], part[:], axis=AX.X, op=ALU.add)
        pfin = psum_pool.tile([P, CB], FP32, tag="ps", name="pfin")
        nc.tensor.matmul(pfin[:1, :1], lhsT=ones[:], rhs=rowtot[:])
        out_sb = epi_pool.tile([1, 1], FP32, tag="osb", name="out_sb")
        nc.vector.tensor_copy(out_sb[:], pfin[:1, :1])
        nc.sync.dma_start(out=out[:, :], in_=out_sb[:])

    with mock.patch("concourse.bacc.get_activation_tables",
                    _filtered_activation_tables):
        nc.compile()
    return nc


_CACHE_LOCK = threading.Lock()
_CACHED_NC = None


def _get_nc():
    global _CACHED_NC
    with _CACHE_LOCK:
        if _CACHED_NC is None:
            _CACHED_NC = _build_kernel()
        return _CACHED_NC


def _run(inputs, trace=False):
    z_i = np.asarray(inputs["z_i"], dtype=np.float32)
    z_j = np.asarray(inputs["z_j"], dtype=np.float32)
    reps = np.concatenate([z_i, z_j], axis=0).astype(ml_dtypes.bfloat16)
    in_maps = [
        {"reps": np.ascontiguousarray(
            np.roll(reps, -ROWS_PER_CORE * i, axis=0))}
        for i in range(N_CORES)
    ]
    nc = _get_nc()
    res = run_bass_kernel_spmd(nc, in_maps, list(range(N_CORES)), trace=trace)
    partials = [float(res.results[i]["out"][0, 0]) for i in range(N_CORES)]
    loss = np.float32(np.sum(np.asarray(partials, dtype=np.float64)) / TWO_N)
    return loss, res


def kernel(**inputs):
    loss, _ = _run(inputs, trace=False)
    return np.asarray(loss, dtype=np.float32)


# revision 21
# speedup vs baseline: 3.8781x; 1.0044x over previous
"""NT-Xent / SimCLR contrastive loss on 8 Trainium2 NeuronCores.

Strategy (data-parallel over rows of the concatenated representations):
  - Host: reps = concat(z_i, z_j) -> [8192, 512], cast bf16. Core i receives
    reps rolled by -1024*i rows so its 1024 rows sit at rows 0..1023 (SPMD
    program identical on every core; positives land at col = row + 4096).
  - Device (per core), fp8 pipeline:
      phase A (per 2048-row group g, pipelined per 1024-row half h):
        load bf16 rows; 1/||row|| via fused square+rowsum (DVE) and
        exp(-0.5*ln(n2/C^2)) (ACT, one table set, = C/||row||); scale rows
        into a permuted fp8e4 staging tile (GPSIMD, strided out AP); store
        2 KiB contiguous runs to DRAM scratch; xbar DMA transpose-load
        [1024,128]u16 -> [128,1024] repsT8 half-tiles. u16 element q of a
        row packs features (2q, 2q+1) = the two DoubleRow fp8 planes.
      phase B (per group nb, m-block): sim slice via DoubleRow fp8 matmuls
        (K=512 as 2 packed 256-chunks, 2x PE rate) into [128, 2048] PSUM;
        ACT computes exp((2/C^2)*sim) with fused row-sum; DVE extracts
        self/positive diagonals with an identity mask + fused reduce.
      A(g) and B(nb=g) are interleaved so ACT/PE/DVE/GPSIMD/DMA pipeline.
      epilogue: denom = rowsum - exp(2*sim_self); partial row loss is
        ln(denom) - 2*pos; partition-sum via a ones-matmul; scalar out.
  - Host: loss = sum(core partials) / 8192.
"""

import math
import sys
import threading
from unittest import mock

sys.path.insert(0, "/opt/trn_rl_repo")

import numpy as np  # noqa: E402
import ml_dtypes  # noqa: E402

import concourse.tile as tile  # noqa: E402
from concourse import bacc, mybir  # noqa: E402
from concourse.bass_utils import run_bass_kernel_spmd  # noqa: E402
from concourse.hw_specs import get_activation_tables  # noqa: E402
from concourse.masks import make_identity  # noqa: E402
from contextlib import ExitStack  # noqa: E402

P = 128
D = 512
TWO_N = 8192
N_CORES = 8
ROWS_PER_CORE = TWO_N // N_CORES  # 1024
T_INV = 2.0  # 1 / temperature (0.5)

NB = 4  # column groups (= row groups in phase A)
CB = TWO_N // NB  # 2048 columns per group / rows per group
TPG = CB // P  # 16 [128, 512] row tiles per group
HB = 2  # halves per group (pipeline granularity)
TPH = TPG // HB  # 8 row tiles per half
HC = CB // HB  # 1024 columns per half
MB = ROWS_PER_CORE // P  # 8 m-blocks of 128 rows per core
NS = CB // 512  # 4 matmul sub-columns of 512 (one PSUM bank each)
KK = 2  # DoubleRow packed K chunks (256 features each)
LOAD_SPLIT = 8  # sub-DMAs per group load (pipeline the first tiles)

C_SCALE = 512.0  # fp8 range scale; rows stored as C*x/||x||
SIM_SCALE = T_INV / (C_SCALE * C_SCALE)  # exact power of two: 2^-17

FP32 = mybir.dt.float32
BF16 = mybir.dt.bfloat16
FP8 = mybir.dt.float8e4
U16 = mybir.dt.uint16
AF = mybir.ActivationFunctionType
ALU = mybir.AluOpType
AX = mybir.AxisListType
DR = mybir.MatmulPerfMode.DoubleRow


def _filtered_activation_tables(arch):
    """Steer every Exp/Ln/Copy activation to the one table set containing
    both Exp and Ln, so the table-load pass cannot thrash between sets."""
    tables = get_activation_tables(arch)
    target = None
    for name, funcs in tables.items():
        if AF.Exp in funcs and AF.Ln in funcs:
            target = name
            break
    if target is None:
        return tables
    steer = {AF.Exp, AF.Ln, AF.Copy, AF.Identity}
    return {
        name: (funcs if name == target else funcs - steer)
        for name, funcs in tables.items()
    }


def _build_kernel():
    nc = bacc.Bacc("TRN2", target_bir_lowering=False, debug=False,
                   num_devices=N_CORES)
    reps = nc.dram_tensor("reps", [TWO_N, D], BF16, kind="ExternalInput").ap()
    out = nc.dram_tensor("out", [1, 1], FP32, kind="ExternalOutput").ap()

    with tile.TileContext(nc) as tc, ExitStack() as ctx:
        rows_pool = ctx.enter_context(tc.tile_pool(name="rows", bufs=1))
        stag_pool = ctx.enter_context(tc.tile_pool(name="stag", bufs=2))
        sq_pool = ctx.enter_context(tc.tile_pool(name="sq", bufs=2))
        stats_pool = ctx.enter_context(tc.tile_pool(name="stats", bufs=1))
        repsT_pool = ctx.enter_context(tc.tile_pool(name="repsT", bufs=1))
        repsT0_pool = ctx.enter_context(tc.tile_pool(name="repsT0", bufs=1))
        dram_pool = ctx.enter_context(
            tc.tile_pool(name="scratch", bufs=KK * NB * HB, space="DRAM"))
        psum_pool = ctx.enter_context(
            tc.tile_pool(name="psum", bufs=2, space="PSUM"))
        exp_pool = ctx.enter_context(tc.tile_pool(name="exp", bufs=3))
        junk_pool = ctx.enter_context(tc.tile_pool(name="junk", bufs=2))
        epi_pool = ctx.enter_context(tc.tile_pool(name="epi", bufs=1))

        # --- constants -----------------------------------------------------
        ident = stats_pool.tile([P, P], FP32, tag="ident", name="ident")
        make_identity(nc, ident[:])
        ones = stats_pool.tile([P, 1], FP32, tag="ones", name="ones")
        nc.gpsimd.memset(ones[:], 1.0)

        # accumulators for the main loop
        rs_all = stats_pool.tile([P, MB * NB], FP32, tag="rs", name="rs_all")
        e_self = stats_pool.tile([P, MB], FP32, tag="eself", name="e_self")
        pos = stats_pool.tile([P, MB], FP32, tag="pos", name="pos")

        # repsT8[kk][g][h]: [128, 1024] u16 - partition p = feature pair
        # (kk*256 + 2p, kk*256 + 2p + 1) packed as 2 fp8 bytes; column q =
        # group row h*1024 + (q%8)*128 + q//8 (scratch permutation). fp8
        # bitcast views give the DoubleRow [128, 2, N] operand APs directly.
        repsT8 = [[[repsT_pool.tile([P, HC], U16, tag=f"rT{kk}_{g}_{h}",
                                    name=f"repsT8_{kk}_{g}_{h}")
                    for h in range(HB)]
                   for g in range(NB)]
                  for kk in range(KK)]

        def rhs_ap(kk, g, ns):
            v = repsT8[kk][g][ns // 2][:].bitcast(FP8).rearrange(
                "p (n two) -> p two n", two=2)
            return v[:, :, (ns % 2) * 512:(ns % 2 + 1) * 512]

        # repsT0[kk]: [128, 2*1024] fp8, plane-slab layout (plane i at cols
        # i*1024..) — LDWEIGHTS rejects the byte-interleaved stride-2 AP, so
        # deinterleave the core's own 1024 columns for the stationary side.
        repsT0 = [repsT0_pool.tile([P, 2 * ROWS_PER_CORE], FP8,
                                   tag=f"rT0_{kk}", name=f"repsT0_{kk}")
                  for kk in range(KK)]

        def lhs_ap(kk, m):
            v = repsT0[kk][:].rearrange("p (two m) -> p two m", two=2)
            return v[:, :, m * P:(m + 1) * P]

        # --- issue all row loads up front (DMA runs ahead) -----------------
        # one tile per (g, h) so consumers wait per-half, not per-group
        # (tile-granular dependency tracking).
        rows_gh = [[rows_pool.tile([P, TPH * D], BF16, tag=f"rows{g}_{h}",
                                   name=f"rows_{g}_{h}")
                    for h in range(HB)]
                   for g in range(NB)]
        SSUB = 2  # sub-DMAs per half-load
        TSUB = TPH // SSUB

        def issue_load(g):
            for h in range(HB):
                for s in range(SSUB):
                    r0 = g * CB + h * HC + s * TSUB * P
                    src = reps[r0:r0 + TSUB * P, :].rearrange(
                        "(t p) d -> p t d", p=P)
                    dst = rows_gh[g][h][
                        :, s * TSUB * D:(s + 1) * TSUB * D].rearrange(
                        "p (t d) -> p t d", d=D)
                    nc.sync.dma_start(out=dst, in_=src)

        # just-in-time: issuing every load up front floods the DMA queues
        # and delays group 0's store+transpose behind 6 MB of prefetch.
        issue_load(0)

        def phase_a(g):
            if g + 1 < NB:
                issue_load(g + 1)
            for h in range(HB):
                rows_h = rows_gh[g][h]
                n2 = stats_pool.tile([P, TPH], FP32, tag="n2", bufs=4,
                                     name=f"n2_{g}_{h}")
                for tl in range(TPH):
                    sq = sq_pool.tile([P, D], BF16, tag="sq",
                                      name=f"sq_{g}_{h}_{tl}")
                    rt = rows_h[:, tl * D:(tl + 1) * D]
                    nc.vector.scalar_tensor_tensor(
                        out=sq[:], in0=rt, scalar=1.0, in1=rt,
                        op0=ALU.mult, op1=ALU.mult,
                        accum_out=n2[:, tl:tl + 1])
                # inv = C * n2^-0.5 = exp(-0.5 * ln(n2 / C^2)); Ln+Exp share
                # one ACT table set (forced via _filtered_activation_tables).
                lnn = stats_pool.tile([P, TPH], FP32, tag="lnn", bufs=4,
                                      name=f"lnn_{g}_{h}")
                nc.scalar.activation(lnn[:], n2[:], AF.Ln,
                                     scale=1.0 / (C_SCALE * C_SCALE))
                inv = stats_pool.tile([P, TPH], FP32, tag="inv", bufs=4,
                                      name=f"inv_{g}_{h}")
                nc.scalar.activation(inv[:], lnn[:], AF.Exp, scale=-0.5)
                # stag: permuted fp8 staging (one tile per half) - byte addr
                # = kk*2048 + t*256 + c - so each kk store is one contiguous
                # 2 KiB run per partition (128 descriptors, not 1024). DVE
                # writes it with a strided out AP (DVE is 1x regardless).
                stag = stag_pool.tile([P, KK * TPH * 256], FP8, tag="stag",
                                      bufs=3, name=f"stag_{g}_{h}")
                sview = stag[:].rearrange("p (kk t c) -> p t kk c",
                                          kk=KK, c=256)
                for tl in range(TPH):
                    src3 = rows_h[:, tl * D:(tl + 1) * D].rearrange(
                        "p (kk c) -> p kk c", kk=KK)
                    nc.vector.tensor_scalar_mul(
                        sview[:, tl], src3, inv[:, tl:tl + 1])
                # store 2 KiB runs, permuted: scratch row q = p*8 + t_local
                # holds half row t_local*128 + p; the transpose-load is then
                # fully contiguous. repsT8 col q <-> half row (q%8)*128+q//8.
                for kk in range(KK):
                    scr = dram_pool.tile([HC, P], U16, tag=f"scr{kk}_{g}_{h}",
                                         name=f"scr_{kk}_{g}_{h}")
                    src = stag[:, kk * TPH * 256:(kk + 1) * TPH * 256]
                    nc.sync.dma_start(
                        out=scr[:].rearrange("(p t) c -> p t c", p=P),
                        in_=src.bitcast(U16).rearrange(
                            "p (t c) -> p t c", c=P))
                    nc.sync.dma_start_transpose(repsT8[kk][g][h][:], scr[:])

        # permuted column q of an et/ps tile holds group row
        # (q//1024)*1024 + (q%8)*128 + (q%1024)//8, so the columns for
        # m-block rows m*128+j (m<8 -> half 0) sit at positions 8*j + m.
        def colsel(ap_2d, m):
            return ap_2d[:, :HC].rearrange("p (j s) -> p s j", s=TPH)[:, m, :]

        def phase_b(nb):
            for m in range(MB):
                ps = psum_pool.tile([P, CB], FP32, tag="ps",
                                    name=f"ps_{nb}_{m}")
                for ns in range(NS):
                    for kk in range(KK):
                        nc.tensor.matmul(
                            ps[:, ns * 512:(ns + 1) * 512],
                            lhsT=lhs_ap(kk, m),
                            rhs=rhs_ap(kk, nb, ns),
                            start=(kk == 0), stop=(kk == KK - 1),
                            perf_mode=DR)
                et = exp_pool.tile([P, CB], BF16, tag="et",
                                   name=f"et_{nb}_{m}")
                nc.scalar.activation(
                    et[:], ps[:], AF.Exp, scale=SIM_SCALE,
                    accum_out=rs_all[:, m * NB + nb:m * NB + nb + 1])
                if nb == 0:
                    # self-similarity diagonal: own row m*128+j sits at
                    # permuted column 8*j + m of half 0.
                    junk = junk_pool.tile([P, P], FP32, tag="junk",
                                          name=f"junk_s_{m}")
                    nc.vector.scalar_tensor_tensor(
                        out=junk[:], in0=colsel(et[:], m),
                        scalar=1.0, in1=ident[:],
                        op0=ALU.mult, op1=ALU.mult,
                        accum_out=e_self[:, m:m + 1])
                if nb == 2:
                    # positive diagonal: global col 4096+row -> group 2,
                    # same permuted position 8*j + m. Read exp(2*s_pos) from
                    # et (not ps!) so PSUM frees as soon as ACT drains it -
                    # otherwise the PE stalls on the ps ring and drops out of
                    # its max p-state. ln() is taken in the epilogue.
                    junk = junk_pool.tile([P, P], FP32, tag="junk",
                                          name=f"junk_p_{m}")
                    nc.vector.scalar_tensor_tensor(
                        out=junk[:], in0=colsel(et[:], m),
                        scalar=1.0, in1=ident[:],
                        op0=ALU.mult, op1=ALU.mult,
                        accum_out=pos[:, m:m + 1])

        # interleave: A(g0) B(0) | A(g1) B(1) | ... so every engine queue
        # pipelines (ACT: ln/exp(g) then 8 exps(nb); PE streams while the
        # next group loads/normalizes/transposes).
        for g in range(NB):
            phase_a(g)
            if g > 0:
                phase_b(g - 1)
            if g == 0:
                # deinterleave + unpermute the core's own 1024 columns into
                # plane-slab lhsT: fp8 index of a half-tile = 2*(8j + s) + i
                # for half row s*128 + j; own rows = half 0 of group 0.
                # ACT does these strided copies: it is idle in the prefix
                # and Copy lives in every table set (no table thrash).
                for kk in range(KK):
                    iv = repsT8[kk][0][0][:].bitcast(FP8).rearrange(
                        "p (j s two) -> p two s j", two=2, s=TPH)
                    ov = repsT0[kk][:].rearrange(
                        "p (two s j) -> p two s j", two=2, s=MB)
                    for i in range(2):
                        nc.scalar.activation(ov[:, i], iv[:, i], AF.Copy)
                phase_b(0)
        phase_b(NB - 1)

        # --- epilogue ------------------------------------------------------
        sums = epi_pool.tile([P, MB], FP32, tag="sums", name="sums")
        nc.vector.tensor_reduce(
            sums[:], rs_all[:].rearrange("p (m b) -> p m b", b=NB),
            axis=AX.X, op=ALU.add)
        denom = epi_pool.tile([P, MB], FP32, tag="denom", name="denom")
        nc.vector.tensor_sub(denom[:], sums[:], e_self[:])
        ld = epi_pool.tile([P, MB], FP32, tag="ld", name="ld")
        nc.scalar.activation(ld[:], denom[:], AF.Ln)
        # partial = ln(denom) - 2*s_pos = ln(denom) - ln(e_pos)
        lde = epi_pool.tile([P, MB], FP32, tag="lde", name="lde")
        nc.scalar.activation(lde[:], pos[:], AF.Ln)
        part = epi_pool.tile([P, MB], FP32, tag="part", name="part")
        nc.vector.tensor_sub(part[:], ld[:], lde[:])
        rowtot = epi_pool.tile([P, 1], FP32, tag="rowtot", name="rowtot")
        nc.vector.tensor_reduce(rowtot[:], part[:], axis=AX.X, op=ALU.add)
        pfin = psum_pool.tile([P, CB], FP32, tag="ps", name="pfin")
        nc.tensor.matmul(pfin[:1, :1], lhsT=ones[:], rhs=rowtot[:])
        out_sb = epi_pool.tile([1, 1], FP32, tag="osb", name="out_sb")
        nc.vector.tensor_copy(out_sb[:], pfin[:1, :1])
        nc.sync.dma_start(out=out[:, :], in_=out_sb[:])

    with mock.patch("concourse.bacc.get_activation_tables",
                    _filtered_activation_tables):
        nc.compile()
    return nc


_CACHE_LOCK = threading.Lock()
_CACHED_NC = None


def _get_nc():
    global _CACHED_NC
    with _CACHE_LOCK:
        if _CACHED_NC is None:
            _CACHED_NC = _build_kernel()
        return _CACHED_NC


def _run(inputs, trace=False):
    z_i = np.asarray(inputs["z_i"], dtype=np.float32)
    z_j = np.asarray(inputs["z_j"], dtype=np.float32)
    reps = np.concatenate([z_i, z_j], axis=0).astype(ml_dtypes.bfloat16)
    in_maps = [
        {"reps": np.ascontiguousarray(
            np.roll(reps, -ROWS_PER_CORE * i, axis=0))}
        for i in range(N_CORES)
    ]
    nc = _get_nc()
    res = run_bass_kernel_spmd(nc, in_maps, list(range(N_CORES)), trace=trace)
    partials = [float(res.results[i]["out"][0, 0]) for i in range(N_CORES)]
    loss = np.float32(np.sum(np.asarray(partials, dtype=np.float64)) / TWO_N)
    return loss, res


def kernel(**inputs):
    loss, _ = _run(inputs, trace=False)
    return np.asarray(loss, dtype=np.float32)


# revision 22
# speedup vs baseline: 3.9440x; 1.0170x over previous
"""NT-Xent / SimCLR contrastive loss on 8 Trainium2 NeuronCores.

Strategy (data-parallel over rows of the concatenated representations):
  - Host: reps = concat(z_i, z_j) -> [8192, 512], cast bf16. Core i receives
    reps rolled by -1024*i rows so its 1024 rows sit at rows 0..1023 (SPMD
    program identical on every core; positives land at col = row + 4096).
  - Device (per core), fp8 pipeline:
      phase A (per 2048-row group g, pipelined per 1024-row half h):
        load bf16 rows; 1/||row|| via fused square+rowsum (DVE) and
        exp(-0.5*ln(n2/C^2)) (ACT, one table set, = C/||row||); scale rows
        into a permuted fp8e4 staging tile (GPSIMD, strided out AP); store
        2 KiB contiguous runs to DRAM scratch; xbar DMA transpose-load
        [1024,128]u16 -> [128,1024] repsT8 half-tiles. u16 element q of a
        row packs features (2q, 2q+1) = the two DoubleRow fp8 planes.
      phase B (per group nb, m-block): sim slice via DoubleRow fp8 matmuls
        (K=512 as 2 packed 256-chunks, 2x PE rate) into [128, 2048] PSUM;
        ACT computes exp((2/C^2)*sim) with fused row-sum; DVE extracts
        self/positive diagonals with an identity mask + fused reduce.
      A(g) and B(nb=g) are interleaved so ACT/PE/DVE/GPSIMD/DMA pipeline.
      epilogue: denom = rowsum - exp(2*sim_self); partial row loss is
        ln(denom) - 2*pos; partition-sum via a ones-matmul; scalar out.
  - Host: loss = sum(core partials) / 8192.
"""

import math
import sys
import threading
from unittest import mock

sys.path.insert(0, "/opt/trn_rl_repo")

import numpy as np  # noqa: E402
import ml_dtypes  # noqa: E402

import concourse.tile as tile  # noqa: E402
from concourse import bacc, mybir  # noqa: E402
from concourse.bass_utils import run_bass_kernel_spmd  # noqa: E402
from concourse.hw_specs import get_activation_tables  # noqa: E402
from concourse.masks import make_identity  # noqa: E402
from contextlib import ExitStack  # noqa: E402

P = 128
D = 512
TWO_N = 8192
N_CORES = 8
ROWS_PER_CORE = TWO_N // N_CORES  # 1024
T_INV = 2.0  # 1 / temperature (0.5)

NB = 4  # column groups (= row groups in phase A)
CB = TWO_N // NB  # 2048 columns per group / rows per group
TPG = CB // P  # 16 [128, 512] row tiles per group
HB = 2  # halves per group (pipeline granularity)
TPH = TPG // HB  # 8 row tiles per half
HC = CB // HB  # 1024 columns per half
MB = ROWS_PER_CORE // P  # 8 m-blocks of 128 rows per core
NS = CB // 512  # 4 matmul sub-columns of 512 (one PSUM bank each)
KK = 2  # DoubleRow packed K chunks (256 features each)
LOAD_SPLIT = 8  # sub-DMAs per group load (pipeline the first tiles)

C_SCALE = 512.0  # fp8 range scale; rows stored as C*x/||x||
SIM_SCALE = T_INV / (C_SCALE * C_SCALE)  # exact power of two: 2^-17

FP32 = mybir.dt.float32
BF16 = mybir.dt.bfloat16
FP8 = mybir.dt.float8e4
U16 = mybir.dt.uint16
AF = mybir.ActivationFunctionType
ALU = mybir.AluOpType
AX = mybir.AxisListType
DR = mybir.MatmulPerfMode.DoubleRow


def _filtered_activation_tables(arch):
    """Steer every Exp/Ln/Copy activation to the one table set containing
    both Exp and Ln, so the table-load pass cannot thrash between sets."""
    tables = get_activation_tables(arch)
    target = None
    for name, funcs in tables.items():
        if AF.Exp in funcs and AF.Ln in funcs:
            target = name
            break
    if target is None:
        return tables
    steer = {AF.Exp, AF.Ln, AF.Copy, AF.Identity}
    return {
        name: (funcs if name == target else funcs - steer)
        for name, funcs in tables.items()
    }


def _build_kernel():
    nc = bacc.Bacc("TRN2", target_bir_lowering=False, debug=False,
                   num_devices=N_CORES)
    reps = nc.dram_tensor("reps", [TWO_N, D], BF16, kind="ExternalInput").ap()
    out = nc.dram_tensor("out", [1, 1], FP32, kind="ExternalOutput").ap()

    with tile.TileContext(nc) as tc, ExitStack() as ctx:
        rows_pool = ctx.enter_context(tc.tile_pool(name="rows", bufs=1))
        stag_pool = ctx.enter_context(tc.tile_pool(name="stag", bufs=2))
        sq_pool = ctx.enter_context(tc.tile_pool(name="sq", bufs=2))
        stats_pool = ctx.enter_context(tc.tile_pool(name="stats", bufs=1))
        repsT_pool = ctx.enter_context(tc.tile_pool(name="repsT", bufs=1))
        repsT0_pool = ctx.enter_context(tc.tile_pool(name="repsT0", bufs=1))
        dram_pool = ctx.enter_context(
            tc.tile_pool(name="scratch", bufs=KK * NB * HB, space="DRAM"))
        psum_pool = ctx.enter_context(
            tc.tile_pool(name="psum", bufs=2, space="PSUM"))
        exp_pool = ctx.enter_context(tc.tile_pool(name="exp", bufs=10))
        junk_pool = ctx.enter_context(tc.tile_pool(name="junk", bufs=2))
        epi_pool = ctx.enter_context(tc.tile_pool(name="epi", bufs=1))

        # --- constants -----------------------------------------------------
        ident = stats_pool.tile([P, P], FP32, tag="ident", name="ident")
        make_identity(nc, ident[:])
        ones = stats_pool.tile([P, 1], FP32, tag="ones", name="ones")
        nc.gpsimd.memset(ones[:], 1.0)

        # accumulators for the main loop
        rs_all = stats_pool.tile([P, MB * NB], FP32, tag="rs", name="rs_all")
        e_self = stats_pool.tile([P, MB], FP32, tag="eself", name="e_self")
        pos = stats_pool.tile([P, MB], FP32, tag="pos", name="pos")

        # repsT8[kk][g][h]: [128, 1024] u16 - partition p = feature pair
        # (kk*256 + 2p, kk*256 + 2p + 1) packed as 2 fp8 bytes; column q =
        # group row h*1024 + (q%8)*128 + q//8 (scratch permutation). fp8
        # bitcast views give the DoubleRow [128, 2, N] operand APs directly.
        repsT8 = [[[repsT_pool.tile([P, HC], U16, tag=f"rT{kk}_{g}_{h}",
                                    name=f"repsT8_{kk}_{g}_{h}")
                    for h in range(HB)]
                   for g in range(NB)]
                  for kk in range(KK)]

        def rhs_ap(kk, g, ns):
            v = repsT8[kk][g][ns // 2][:].bitcast(FP8).rearrange(
                "p (n two) -> p two n", two=2)
            return v[:, :, (ns % 2) * 512:(ns % 2 + 1) * 512]

        # repsT0[kk]: [128, 2*1024] fp8, plane-slab layout (plane i at cols
        # i*1024..) — LDWEIGHTS rejects the byte-interleaved stride-2 AP, so
        # deinterleave the core's own 1024 columns for the stationary side.
        repsT0 = [repsT0_pool.tile([P, 2 * ROWS_PER_CORE], FP8,
                                   tag=f"rT0_{kk}", name=f"repsT0_{kk}")
                  for kk in range(KK)]

        def lhs_ap(kk, m):
            v = repsT0[kk][:].rearrange("p (two m) -> p two m", two=2)
            return v[:, :, m * P:(m + 1) * P]

        # --- issue all row loads up front (DMA runs ahead) -----------------
        # one tile per (g, h) so consumers wait per-half, not per-group
        # (tile-granular dependency tracking).
        rows_gh = [[rows_pool.tile([P, TPH * D], BF16, tag=f"rows{g}_{h}",
                                   name=f"rows_{g}_{h}")
                    for h in range(HB)]
                   for g in range(NB)]
        SSUB = 2  # sub-DMAs per half-load
        TSUB = TPH // SSUB

        def issue_load(g):
            for h in range(HB):
                for s in range(SSUB):
                    r0 = g * CB + h * HC + s * TSUB * P
                    src = reps[r0:r0 + TSUB * P, :].rearrange(
                        "(t p) d -> p t d", p=P)
                    dst = rows_gh[g][h][
                        :, s * TSUB * D:(s + 1) * TSUB * D].rearrange(
                        "p (t d) -> p t d", d=D)
                    nc.sync.dma_start(out=dst, in_=src)

        # just-in-time: issuing every load up front floods the DMA queues
        # and delays group 0's store+transpose behind 6 MB of prefetch.
        issue_load(0)

        def phase_a(g):
            if g + 1 < NB:
                issue_load(g + 1)
            for h in range(HB):
                rows_h = rows_gh[g][h]
                n2 = stats_pool.tile([P, TPH], FP32, tag="n2", bufs=4,
                                     name=f"n2_{g}_{h}")
                for tl in range(TPH):
                    sq = sq_pool.tile([P, D], BF16, tag="sq",
                                      name=f"sq_{g}_{h}_{tl}")
                    rt = rows_h[:, tl * D:(tl + 1) * D]
                    nc.vector.scalar_tensor_tensor(
                        out=sq[:], in0=rt, scalar=1.0, in1=rt,
                        op0=ALU.mult, op1=ALU.mult,
                        accum_out=n2[:, tl:tl + 1])
                # inv = C * n2^-0.5 = exp(-0.5 * ln(n2 / C^2)); Ln+Exp share
                # one ACT table set (forced via _filtered_activation_tables).
                lnn = stats_pool.tile([P, TPH], FP32, tag="lnn", bufs=4,
                                      name=f"lnn_{g}_{h}")
                nc.scalar.activation(lnn[:], n2[:], AF.Ln,
                                     scale=1.0 / (C_SCALE * C_SCALE))
                inv = stats_pool.tile([P, TPH], FP32, tag="inv", bufs=4,
                                      name=f"inv_{g}_{h}")
                nc.scalar.activation(inv[:], lnn[:], AF.Exp, scale=-0.5)
                # stag: permuted fp8 staging (one tile per half) - byte addr
                # = kk*2048 + t*256 + c - so each kk store is one contiguous
                # 2 KiB run per partition (128 descriptors, not 1024). DVE
                # writes it with a strided out AP (DVE is 1x regardless).
                stag = stag_pool.tile([P, KK * TPH * 256], FP8, tag="stag",
                                      bufs=3, name=f"stag_{g}_{h}")
                sview = stag[:].rearrange("p (kk t c) -> p t kk c",
                                          kk=KK, c=256)
                for tl in range(TPH):
                    src3 = rows_h[:, tl * D:(tl + 1) * D].rearrange(
                        "p (kk c) -> p kk c", kk=KK)
                    nc.vector.tensor_scalar_mul(
                        sview[:, tl], src3, inv[:, tl:tl + 1])
                # store 2 KiB runs, permuted: scratch row q = p*8 + t_local
                # holds half row t_local*128 + p; the transpose-load is then
                # fully contiguous. repsT8 col q <-> half row (q%8)*128+q//8.
                for kk in range(KK):
                    scr = dram_pool.tile([HC, P], U16, tag=f"scr{kk}_{g}_{h}",
                                         name=f"scr_{kk}_{g}_{h}")
                    src = stag[:, kk * TPH * 256:(kk + 1) * TPH * 256]
                    nc.sync.dma_start(
                        out=scr[:].rearrange("(p t) c -> p t c", p=P),
                        in_=src.bitcast(U16).rearrange(
                            "p (t c) -> p t c", c=P))
                    nc.sync.dma_start_transpose(repsT8[kk][g][h][:], scr[:])

        # permuted column q of an et/ps tile holds group row
        # (q//1024)*1024 + (q%8)*128 + (q%1024)//8, so the columns for
        # m-block rows m*128+j (m<8 -> half 0) sit at positions 8*j + m.
        def colsel(ap_2d, m):
            return ap_2d[:, :HC].rearrange("p (j s) -> p s j", s=TPH)[:, m, :]

        def phase_b(nb):
            for m in range(MB):
                ps = psum_pool.tile([P, CB], FP32, tag="ps",
                                    name=f"ps_{nb}_{m}")
                for ns in range(NS):
                    for kk in range(KK):
                        nc.tensor.matmul(
                            ps[:, ns * 512:(ns + 1) * 512],
                            lhsT=lhs_ap(kk, m),
                            rhs=rhs_ap(kk, nb, ns),
                            start=(kk == 0), stop=(kk == KK - 1),
                            perf_mode=DR)
                et = exp_pool.tile([P, CB], BF16, tag="et",
                                   name=f"et_{nb}_{m}")
                nc.scalar.activation(
                    et[:], ps[:], AF.Exp, scale=SIM_SCALE,
                    accum_out=rs_all[:, m * NB + nb:m * NB + nb + 1])
                if nb == 0:
                    # self-similarity diagonal: own row m*128+j sits at
                    # permuted column 8*j + m of half 0.
                    junk = junk_pool.tile([P, P], FP32, tag="junk",
                                          name=f"junk_s_{m}")
                    nc.vector.scalar_tensor_tensor(
                        out=junk[:], in0=colsel(et[:], m),
                        scalar=1.0, in1=ident[:],
                        op0=ALU.mult, op1=ALU.mult,
                        accum_out=e_self[:, m:m + 1])
                if nb == 2:
                    # positive diagonal: global col 4096+row -> group 2,
                    # same permuted position 8*j + m. Read exp(2*s_pos) from
                    # et (not ps!) so PSUM frees as soon as ACT drains it -
                    # otherwise the PE stalls on the ps ring and drops out of
                    # its max p-state. ln() is taken in the epilogue.
                    junk = junk_pool.tile([P, P], FP32, tag="junk",
                                          name=f"junk_p_{m}")
                    nc.vector.scalar_tensor_tensor(
                        out=junk[:], in0=colsel(et[:], m),
                        scalar=1.0, in1=ident[:],
                        op0=ALU.mult, op1=ALU.mult,
                        accum_out=pos[:, m:m + 1])

        # interleave: A(g0) B(0) | A(g1) B(1) | ... so every engine queue
        # pipelines (ACT: ln/exp(g) then 8 exps(nb); PE streams while the
        # next group loads/normalizes/transposes).
        for g in range(NB):
            phase_a(g)
            if g > 0:
                phase_b(g - 1)
            if g == 0:
                # deinterleave + unpermute the core's own 1024 columns into
                # plane-slab lhsT: fp8 index of a half-tile = 2*(8j + s) + i
                # for half row s*128 + j; own rows = half 0 of group 0.
                # ACT does these strided copies: it is idle in the prefix
                # and Copy lives in every table set (no table thrash). The
                # first two m-blocks are copied first to unblock matmul m=0.
                for s0, s1 in ((0, 2), (2, MB)):
                    for kk in range(KK):
                        iv = repsT8[kk][0][0][:].bitcast(FP8).rearrange(
                            "p (j s two) -> p two s j", two=2, s=TPH)
                        ov = repsT0[kk][:].rearrange(
                            "p (two s j) -> p two s j", two=2, s=MB)
                        for i in range(2):
                            nc.scalar.activation(
                                ov[:, i, s0:s1], iv[:, i, s0:s1], AF.Copy)
                phase_b(0)
        phase_b(NB - 1)

        # --- epilogue ------------------------------------------------------
        sums = epi_pool.tile([P, MB], FP32, tag="sums", name="sums")
        nc.vector.tensor_reduce(
            sums[:], rs_all[:].rearrange("p (m b) -> p m b", b=NB),
            axis=AX.X, op=ALU.add)
        denom = epi_pool.tile([P, MB], FP32, tag="denom", name="denom")
        nc.vector.tensor_sub(denom[:], sums[:], e_self[:])
        ld = epi_pool.tile([P, MB], FP32, tag="ld", name="ld")
        nc.scalar.activation(ld[:], denom[:], AF.Ln)
        # partial = ln(denom) - 2*s_pos = ln(denom) - ln(e_pos)
        lde = epi_pool.tile([P, MB], FP32, tag="lde", name="lde")
        nc.scalar.activation(lde[:], pos[:], AF.Ln)
        part = epi_pool.tile([P, MB], FP32, tag="part", name="part")
        nc.vector.tensor_sub(part[:], ld[:], lde[:])
        rowtot = epi_pool.tile([P, 1], FP32, tag="rowtot", name="rowtot")
        nc.vector.tensor_reduce(rowtot[:], part[:], axis=AX.X, op=ALU.add)
        pfin = psum_pool.tile([P, CB], FP32, tag="ps", name="pfin")
        nc.tensor.matmul(pfin[:1, :1], lhsT=ones[:], rhs=rowtot[:])
        out_sb = epi_pool.tile([1, 1], FP32, tag="osb", name="out_sb")
        nc.vector.tensor_copy(out_sb[:], pfin[:1, :1])
        nc.sync.dma_start(out=out[:, :], in_=out_sb[:])

    with mock.patch("concourse.bacc.get_activation_tables",
                    _filtered_activation_tables):
        nc.compile()
    return nc


_CACHE_LOCK = threading.Lock()
_CACHED_NC = None


def _get_nc():
    global _CACHED_NC
    with _CACHE_LOCK:
        if _CACHED_NC is None:
            _CACHED_NC = _build_kernel()
        return _CACHED_NC


def _run(inputs, trace=False):
    z_i = np.asarray(inputs["z_i"], dtype=np.float32)
    z_j = np.asarray(inputs["z_j"], dtype=np.float32)
    reps = np.concatenate([z_i, z_j], axis=0).astype(ml_dtypes.bfloat16)
    in_maps = [
        {"reps": np.ascontiguousarray(
            np.roll(reps, -ROWS_PER_CORE * i, axis=0))}
        for i in range(N_CORES)
    ]
    nc = _get_nc()
    res = run_bass_kernel_spmd(nc, in_maps, list(range(N_CORES)), trace=trace)
    partials = [float(res.results[i]["out"][0, 0]) for i in range(N_CORES)]
    loss = np.float32(np.sum(np.asarray(partials, dtype=np.float64)) / TWO_N)
    return loss, res


def kernel(**inputs):
    loss, _ = _run(inputs, trace=False)
    return np.asarray(loss, dtype=np.float32)
